# revision 35
# baseline (speedup 1.0000x reference)
"""Causal attention head on 8 trn2 NeuronCores.

Sharding: core c = (batch b = c//2, type t = c%2). Each core handles 4
query stripes of 512 of its batch. Causal balance: type A gets stripes
[7,5,2,0] with real key-block counts R_A=[32,24,12,4]; type B stripes
[6,4,3,1] with R_B=[28,20,16,8]. One SPMD program: every core runs the
padded template T=[32,24,16,8]; per-core behaviour comes only from input
data (per-core threshold scalars select ones/triangle/zero mask tiles).

Everything on the PE array is bf16. Score matmuls pack two key-blocks
per issue via PE row tiling: kT pairs live on partition halves 0:64 /
64:128 (host interleaves ek into even/odd block regions), qT is
duplicated onto both halves by a column-duplicated Wq.

Attention processes pairs (2 key-blocks) in GROUPS of 2 to amortize the
PE stationary-switch stall (64-row score tiles vs 128-row pv tiles cost
~106ns per transition: drain + non-overlapped LDWEIGHTS). Per group g:
pv matmuls for group g-2 (8 MMs), then scores for the 2 pairs of group
g (psc double-buffer holds exactly 2 pairs), exp per pair on ACT, mask
mul (DVE) on the last-4 pairs, acc += e (DVE fp16).

The emission is software-pipelined ACROSS slots so the PE never idles at
a slot boundary: after slot j's last score group we emit [proj(p+1),
mask-gen(slot j+1), pv(ng-2)], and slot j+1's first two score groups
(which have no pv partner: GLAG=2) interleave with slot j's final pv
drain group and output copies. Input DMAs are split across the SP and
ACT issue queues with the critical slot-3 set (th/qk/wk/wq/ek/eq)
first; dummy matmuls keep the PE busy under the DMA wait so the HAM
clock-gate opens (1.2->2.4 GHz) early. QK (query-index minus key-index
iota) comes in as a host input so slot-3 masks only wait on two small
DMAs. Outputs: outT bf16 [256,2048] (unnormalized; o0 drained via DVE,
o1 via ACT in parallel), acc fp16 [128,4096]. Host: r = colsum(acc)
folded over pair halves; out = (outT/r).T.
"""

import sys

sys.path.insert(0, "/opt/trn_rl_repo")

import numpy as np
import ml_dtypes

B, S, DM, DQ = 4, 4096, 256, 64
T = [32, 24, 16, 8]  # padded template: key-blocks per slot
STRIPES_A = [7, 5, 2, 0]  # R_A = [32, 24, 12, 4]
STRIPES_B = [6, 4, 3, 1]  # R_B = [28, 20, 16, 8]
# Per-pair mask thresholds for the last 4 pairs of a slot (f16-safe:
# QK values lie in [-255, 511], so +-1000 mean all-ones / all-zeros).
# exact slot (R == T): pairs are [ones, ones, tri(0/128), tri(256/384)]
# padded slot (R == T-4): pairs are [tri(0/128), tri(256/384), zero, zero]
TH_EXACT = [-1000.0, -1000.0, 0.0, 256.0]
TH_PAD = [0.0, 256.0, 1000.0, 1000.0]

_CACHE = {}


def _build_nc():
    import concourse.bass as bass  # noqa: F401
    import concourse.tile as tile
    from concourse import bacc, mybir

    dt = mybir.dt
    f32, bf, f16 = dt.float32, dt.bfloat16, dt.float16

    nc = bacc.Bacc(
        "TRN2",
        target_bir_lowering=False,
        debug=False,
        enable_asserts=False,
        num_devices=8,
    )

    def din(name, shape, d):
        return nc.dram_tensor(name, shape, d, kind="ExternalInput").ap()

    eq = din("eq", [256, 2048], bf)
    ek = din("ek", [256, 4096], bf)  # column-reordered: even blocks, then odd
    ev = din("ev", [256, 4096], bf)
    # all projection weights in one tensor (one DMA per 128-partition half):
    # cols 0:64 Wk.T, 64:192 Wq.T duplicated, 192:448 Wv.T
    wkqv = din("wkqv", [256, 448], bf)
    # thresholds + QK iota in one f16 tensor: cols 0:16 th, 16:1040 QK
    # (QK[p, i2*512+qi] = qi-128*i2-p; th in {-1000,0,256,1000})
    thqk = din("thqk", [128, 1040], f16)
    outT = nc.dram_tensor("outT", [256, 2048], bf, kind="ExternalOutput").ap()
    acc_out = nc.dram_tensor("acc", [128, 4096], f16, kind="ExternalOutput").ap()

    Exp = mybir.ActivationFunctionType.Exp
    GE = mybir.AluOpType.is_ge

    with tile.TileContext(nc) as tc:
        from contextlib import ExitStack

        with ExitStack() as ctx:
            const = ctx.enter_context(tc.tile_pool(name="const", bufs=1))

            # ---- persistent SBUF tensors ----
            eq_sb = [const.tile([128, 2048], bf, tag=f"eq{h}", name=f"eq{h}") for h in range(2)]
            ek_sb = [const.tile([128, 4096], bf, tag=f"ek{h}", name=f"ek{h}") for h in range(2)]
            ev_sb = [const.tile([128, 4096], bf, tag=f"ev{h}", name=f"ev{h}") for h in range(2)]
            wkqv_sb = const.tile([128, 896], bf, tag="wkqv", name="wkqv")
            # per-half weight views into wkqv_sb
            wk_h = [wkqv_sb[:, h * 448 : h * 448 + 64] for h in range(2)]
            wq_h = [wkqv_sb[:, h * 448 + 64 : h * 448 + 192] for h in range(2)]
            wv_h = [wkqv_sb[:, h * 448 + 192 : h * 448 + 448] for h in range(2)]
            thqk_sb = const.tile([128, 1040], f16, tag="thqk", name="thqk")
            qk = thqk_sb[:, 16:1040]
            th_sb = const.tile([128, 16], f32, tag="th32", name="th32")
            qT = const.tile([128, 2048], bf, tag="qT", name="qT")  # dup halves
            kTp = const.tile([128, 2048], bf, tag="kTp", name="kTp")  # pair-packed
            v_sb = const.tile([128, 32 * 256], bf, tag="v", name="v")
            acc = const.tile([128, 4096], f16, tag="acc", name="acc")
            mk = const.tile([128, 16 * 1024], bf, tag="mk", name="mk")

            # Input DMAs are issued from both SP and Activation queues (half
            # each) and staged per phase: phases 0-1 up front, later phases
            # from inside the pipeline so issue time hides under compute.
            # Phase-0 order is criticality-driven: th/qk (mask gen), wk/wq,
            # ek (kTp proj), eq (qT proj), wv, ev.
            def dma_phase(p, j):
                # phases 0-1 are issued up front across FOUR queues (queue
                # issue+transfer serialize at ~1us per 128KB on one queue);
                # later phases go all-SP so ACT/DVE stay free for compute.
                cs_q = slice(j * 512, (j + 1) * 512)
                if p == 0:
                    nc.scalar.dma_start(thqk_sb[:], thqk[:])
                for reg in range(2):  # 0: even region, 1: odd region
                    cs = slice(reg * 2048 + p * 512, reg * 2048 + (p + 1) * 512)
                    if p == 0:
                        nc.sync.dma_start(ek_sb[0][:, cs], ek[0:128, cs])
                        nc.gpsimd.dma_start(ek_sb[1][:, cs], ek[128:256, cs])
                    elif p == 1:
                        nc.gpsimd.dma_start(ek_sb[0][:, cs], ek[0:128, cs])
                        nc.gpsimd.dma_start(ek_sb[1][:, cs], ek[128:256, cs])
                    else:
                        nc.sync.dma_start(ek_sb[0][:, cs], ek[0:128, cs])
                        nc.sync.dma_start(ek_sb[1][:, cs], ek[128:256, cs])
                if p == 0:
                    nc.sync.dma_start(wkqv_sb[:, 0:448], wkqv[0:128, :])
                    nc.scalar.dma_start(wkqv_sb[:, 448:896], wkqv[128:256, :])
                alt = nc.scalar if p == 0 else nc.sync
                nc.sync.dma_start(eq_sb[0][:, cs_q], eq[0:128, cs_q])
                alt.dma_start(eq_sb[1][:, cs_q], eq[128:256, cs_q])
                for cc in EV_CH[p]:
                    cs = slice(cc * 512, (cc + 1) * 512)
                    nc.sync.dma_start(ev_sb[0][:, cs], ev[0:128, cs])
                    alt.dma_start(ev_sb[1][:, cs], ev[128:256, cs])

            PHASES = ((0, 3), (1, 2), (2, 1), (3, 0))
            V_QUOTA = ((0, 1, 2, 3, 4, 5), (6, 7, 8, 9), (10, 11, 12, 13), (14, 15))
            EV_CH = ((0, 1, 2), (3, 4), (5, 6), (7,))
            # scr memset first: it must sit ahead of the phase-1 ek DMAs in
            # the GpSimd FIFO or the PE warm-up stalls behind 4 transfers
            scr = const.tile([128, 512], bf, tag="scr", name="scr")
            nc.gpsimd.memset(scr[:], 0.0)
            dma_phase(0, 3)
            dma_phase(1, 2)

            def mask_gen(j):
                # mk[t] = (QK >= th[t]) in bf16, on DVE (~478ns each; GpSimd's
                # tensor_scalar ucode is 30x slower for this op, do NOT use it)
                for m in range(4):
                    t = j * 4 + m
                    nc.vector.tensor_scalar(
                        mk[:, t * 1024 : (t + 1) * 1024], qk[:], th_sb[:, t : t + 1], None, GE
                    )

            # Slot-3 masks up front (its first score groups are all masked);
            # slot j+1's masks are generated during phase p's tail so the
            # 478ns-each DVE ops never sit ahead of critical qT/kTp casts.
            nc.vector.tensor_copy(th_sb[:], thqk_sb[:, 0:16])  # f16 -> f32
            mask_gen(3)

            pp = ctx.enter_context(tc.tile_pool(name="pp", bufs=2, space="PSUM"))
            psc = ctx.enter_context(tc.tile_pool(name="psc", bufs=2, space="PSUM"))
            po_pool = ctx.enter_context(tc.tile_pool(name="po", bufs=1, space="PSUM"))
            epool = ctx.enter_context(tc.tile_pool(name="e", bufs=10))
            opool = ctx.enter_context(tc.tile_pool(name="o", bufs=2))

            for _ in range(6):
                ps = pp.tile([128, 512], f32, tag="ps", name="ps")
                nc.tensor.matmul(ps[:], scr[:, 0:128], scr[:], start=True, stop=True)

            def proj_qk(p, j):
                # kTp chunk p: even blocks -> partitions 0:64, odd -> 64:128
                # (the two 64-partition outputs col-tile and run concurrent)
                ps = pp.tile([128, 512], f32, tag="ps", name="ps")
                for half in range(2):
                    dst = ps[half * 64 : (half + 1) * 64, :]
                    for h in range(2):
                        nc.tensor.matmul(
                            dst,
                            wk_h[h],
                            ek_sb[h][:, half * 2048 + p * 512 : half * 2048 + (p + 1) * 512],
                            start=(h == 0),
                            stop=(h == 1),
                        )
                nc.vector.tensor_copy(kTp[:, p * 512 : (p + 1) * 512], ps[:])
                # qT chunk j (duplicated onto both halves by the dup'd wq)
                ps = pp.tile([128, 512], f32, tag="ps", name="ps")
                for h in range(2):
                    nc.tensor.matmul(
                        ps[:],
                        wq_h[h],
                        eq_sb[h][:, j * 512 : (j + 1) * 512],
                        start=(h == 0),
                        stop=(h == 1),
                    )
                nc.vector.tensor_copy(qT[:, j * 512 : (j + 1) * 512], ps[:])

            def v_proj(tiles):
                # v pairs (natural [keys, 256] bf16); copies go 2-of-3 on
                # DVE, 1-of-3 on ACT so ACT keeps the exp cadence
                for n, i in enumerate(tiles):
                    ps = pp.tile([128, 512], f32, tag="ps", name="ps")
                    for s in range(2):
                        t = 2 * i + s
                        for h in range(2):
                            nc.tensor.matmul(
                                ps[:, s * 256 : (s + 1) * 256],
                                ev_sb[h][:, t * 128 : (t + 1) * 128],
                                wv_h[h],
                                start=(h == 0),
                                stop=(h == 1),
                            )
                    dst = v_sb[:, i * 512 : (i + 1) * 512]
                    # 1-of-3 on ACT: more ACT copies break the back-to-back
                    # exp pipelining, more DVE copies choke the acc chain
                    if n % 3 == 2:
                        nc.scalar.copy(dst, ps[:])
                    else:
                        nc.vector.tensor_copy(dst, ps[:])

            G, GLAG = 2, 2  # score-pair group size, pv lag in groups

            class Slot:
                def __init__(self, p, j):
                    self.p, self.j = p, j
                    self.npairs = T[j] // 2
                    self.ngroups = self.npairs // G
                    self.po0 = po_pool.tile([128, 512], f32, tag="po0", name="po0")
                    self.po1 = po_pool.tile([128, 512], f32, tag="po1", name="po1")
                    self.qs_top = qT[0:64, j * 512 : (j + 1) * 512]
                    self.qs_bot = qT[64:128, j * 512 : (j + 1) * 512]
                    self.acc_j = acc[:, j * 1024 : (j + 1) * 1024]
                    self.es = [None] * self.npairs
                    # Process masked pairs right after the pipeline fills
                    # (their DVE mask-muls get full lag slack) and end every
                    # slot on unmasked pairs so the drain has no DVE dep.
                    np_ = self.npairs
                    if np_ > 4:
                        k = min(4, np_ - 8)
                        self.proc = (
                            list(range(k)) + list(range(np_ - 4, np_)) + list(range(k, np_ - 4))
                        )
                    else:
                        self.proc = list(range(np_))

                def pv_group(self, h):
                    # 4 pv matmuls per pair, 8 per group; LDWEIGHTS chains
                    # through the background weight buffer at 216ns cadence.
                    # (A 64-row split onto the two row-tiled pipes with
                    # interleaved po0/po1 accumulation hung the device --
                    # concurrent accumulation into one PSUM bank from two
                    # row-groups is NOT safe in practice. Do not retry.)
                    for dd in range(h * G, h * G + G):
                        e = self.es[dd]
                        for half in range(2):
                            u = 2 * self.proc[dd] + half
                            eh = e[:, half * 512 : (half + 1) * 512]
                            first = dd == 0 and half == 0
                            last = dd == self.npairs - 1 and half == 1
                            nc.tensor.matmul(
                                self.po0[:],
                                v_sb[:, u * 256 : u * 256 + 128],
                                eh,
                                start=first,
                                stop=last,
                            )
                            nc.tensor.matmul(
                                self.po1[:],
                                v_sb[:, u * 256 + 128 : (u + 1) * 256],
                                eh,
                                start=first,
                                stop=last,
                            )

                def sc_group(self, g):
                    # scores for the G pairs of group g, then exp/mask/acc
                    for d in range(g * G, g * G + G):
                        tp = self.proc[d]
                        ps = psc.tile([128, 1024], f32, tag="ps", name="ps")
                        nc.tensor.matmul(
                            ps[:, 0:512],
                            kTp[0:64, tp * 128 : (tp + 1) * 128],
                            self.qs_top,
                            start=True,
                            stop=True,
                        )
                        nc.tensor.matmul(
                            ps[:, 512:1024],
                            kTp[64:128, tp * 128 : (tp + 1) * 128],
                            self.qs_bot,
                            start=True,
                            stop=True,
                        )
                        e = epool.tile([128, 1024], bf, tag="e", name="e")
                        nc.scalar.activation(e[:], ps[:], Exp, scale=0.125)
                        if tp >= self.npairs - 4:
                            t = self.j * 4 + tp - (self.npairs - 4)
                            nc.vector.tensor_mul(
                                e[:], e[:], mk[:, t * 1024 : (t + 1) * 1024]
                            )
                        if d == 0:
                            nc.vector.tensor_copy(self.acc_j, e[:])
                        else:
                            nc.vector.tensor_add(self.acc_j, self.acc_j, e[:])
                        self.es[d] = e

                def fin(self, last_phase):
                    nc.sync.dma_start(
                        acc_out[:, self.j * 1024 : (self.j + 1) * 1024], self.acc_j
                    )
                    o0 = opool.tile([128, 512], bf, tag="o0", name="o0")
                    o1 = opool.tile([128, 512], bf, tag="o1", name="o1")
                    # o0 via DVE, o1 via ACT: the drains run in parallel and
                    # ACT never blocks long behind the pv-drain semaphore.
                    nc.vector.tensor_copy(o0[:], self.po0[:])
                    nc.scalar.copy(o1[:], self.po1[:])
                    j = self.j
                    nc.sync.dma_start(outT[0:128, j * 512 : (j + 1) * 512], o0[:])
                    if last_phase:  # final slot: second out DMA from ACT queue
                        nc.scalar.dma_start(outT[128:256, j * 512 : (j + 1) * 512], o1[:])
                    else:
                        nc.sync.dma_start(outT[128:256, j * 512 : (j + 1) * 512], o1[:])

            # ---- software-pipelined emission across phases ----
            # Per phase p (slot j): [sc0, prev.pv(ng-1), sc1, prev.fin,
            # v_proj(p), {pv(g-2), sc(g)}..., proj(p+1), mask_gen(j+1),
            # pv(ng-2)]; the final pv drain and output copies of slot j land
            # inside phase p+1's head, overlapped with its first exps.
            proj_qk(0, 3)
            prev = None
            for p, j in PHASES:
                s = Slot(p, j)
                s.sc_group(0)
                if prev is not None:
                    prev.pv_group(prev.ngroups - 1)
                s.sc_group(1)
                if prev is not None:
                    prev.fin(last_phase=False)
                # phase 0: only tiles 0-1 are needed before pv(0); tiles 2-3
                # are emitted in the tail so the first exps start sooner
                v_proj(V_QUOTA[p][:2] if p == 0 else V_QUOTA[p])
                for g in range(2, s.ngroups):
                    s.pv_group(g - 2)
                    s.sc_group(g)
                if p < 3:
                    proj_qk(p + 1, PHASES[p + 1][1])
                    mask_gen(PHASES[p + 1][1])
                if p == 0:
                    v_proj(V_QUOTA[0][2:])
                s.pv_group(s.ngroups - 2)
                if p + 2 <= 3:
                    dma_phase(*PHASES[p + 2])
                prev = s
            prev.pv_group(prev.ngroups - 1)
            prev.fin(last_phase=True)

    nc.compile()
    return nc


def kernel(encodings_for_q, encodings_for_k, encodings_for_v, mask, Wq, Wk, Wv):
    from concourse.bass_utils import run_bass_kernel_spmd

    if "nc" not in _CACHE:
        _CACHE["nc"] = _build_nc()
    nc = _CACHE["nc"]

    bf = ml_dtypes.bfloat16
    # [Wk.T | Wq.T dup | Wv.T] -> [256, 448]
    wkqv_host = np.ascontiguousarray(
        np.concatenate([Wk.T, Wq.T, Wq.T, Wv.T], axis=1), dtype=bf
    )
    # QK[p, i2*512 + qi] = qi - 128*i2 - p
    qi = np.arange(512, dtype=np.float32)
    i2 = np.arange(2, dtype=np.float32)
    pch = np.arange(128, dtype=np.float32)
    qk_host = (
        (qi[None, None, :] - 128 * i2[None, :, None] - pch[:, None, None])
        .reshape(128, 1024)
        .astype(np.float16)
    )

    in_maps = []
    metas = []
    for c in range(8):
        b, t = c // 2, c % 2
        stripes = STRIPES_A if t == 0 else STRIPES_B
        eqT = np.concatenate(
            [encodings_for_q[b, st * 512 : (st + 1) * 512, :].T for st in stripes],
            axis=1,
        )
        ekT = encodings_for_k[b].T.reshape(256, 32, 128)
        ek_reord = np.concatenate([ekT[:, 0::2, :], ekT[:, 1::2, :]], axis=1).reshape(
            256, 4096
        )
        # thresholds: slot j exact if R[j] == T[j]
        thr = np.empty((16,), dtype=np.float16)
        for j in range(4):
            R = 4 * (stripes[j] + 1)
            vals = TH_EXACT if R == T[j] else TH_PAD
            thr[j * 4 : (j + 1) * 4] = vals
        thqk_host = np.ascontiguousarray(
            np.concatenate([np.broadcast_to(thr, (128, 16)), qk_host], axis=1),
            dtype=np.float16,
        )
        in_maps.append(
            {
                "eq": np.ascontiguousarray(eqT, dtype=bf),
                "ek": np.ascontiguousarray(ek_reord, dtype=bf),
                "ev": np.ascontiguousarray(encodings_for_v[b].T, dtype=bf),
                "wkqv": wkqv_host,
                "thqk": thqk_host,
            }
        )
        metas.append((b, stripes))

    res = run_bass_kernel_spmd(nc, in_maps, core_ids=list(range(8)))
    _CACHE["last_res"] = res

    out = np.empty((B, S, DM), dtype=np.float32)
    for c in range(8):
        b, stripes = metas[c]
        oT = res.results[c]["outT"].astype(np.float32)
        a = res.results[c]["acc"].astype(np.float32)
        for j, st in enumerate(stripes):
            r = a[:, j * 1024 : j * 1024 + 512].sum(0) + a[
                :, j * 1024 + 512 : (j + 1) * 1024
            ].sum(0)
            blk = oT[:, j * 512 : (j + 1) * 512] / r[None, :]
            out[b, st * 512 : (st + 1) * 512, :] = blk.T
    return out


# revision 41
# speedup vs baseline: 1.0029x; 1.0029x over previous
"""Causal attention head on 8 trn2 NeuronCores.

Sharding: core c = (batch b = c//2, type t = c%2). Each core handles 4
query stripes of 512 of its batch. Causal balance: type A gets stripes
[7,5,2,0] with real key-block counts R_A=[32,24,12,4]; type B stripes
[6,4,3,1] with R_B=[28,20,16,8]. One SPMD program: every core runs the
padded template T=[32,24,16,8]; per-core behaviour comes only from input
data (per-core threshold scalars select ones/triangle/zero mask tiles).

Everything on the PE array is bf16. Score matmuls pack two key-blocks
per issue via PE row tiling: kT pairs live on partition halves 0:64 /
64:128 (host interleaves ek into even/odd block regions), qT is
duplicated onto both halves by a column-duplicated Wq.

Attention processes pairs (2 key-blocks) in GROUPS of 2 to amortize the
PE stationary-switch stall (64-row score tiles vs 128-row pv tiles cost
~106ns per transition: drain + non-overlapped LDWEIGHTS). Per group g:
pv matmuls for group g-2 (8 MMs), then scores for the 2 pairs of group
g (psc double-buffer holds exactly 2 pairs), exp per pair on ACT, mask
mul (DVE) on the last-4 pairs, acc += e (DVE fp16).

The emission is software-pipelined ACROSS slots so the PE never idles at
a slot boundary: after slot j's last score group we emit [proj(p+1),
mask-gen(slot j+1), pv(ng-2)], and slot j+1's first two score groups
(which have no pv partner: GLAG=2) interleave with slot j's final pv
drain group and output copies. Input DMAs are split across the SP and
ACT issue queues with the critical slot-3 set (th/qk/wk/wq/ek/eq)
first; dummy matmuls keep the PE busy under the DMA wait so the HAM
clock-gate opens (1.2->2.4 GHz) early. QK (query-index minus key-index
iota) comes in as a host input so slot-3 masks only wait on two small
DMAs. Outputs: outT bf16 [256,2048] (unnormalized; o0 drained via DVE,
o1 via ACT in parallel), acc fp16 [128,4096]. Host: r = colsum(acc)
folded over pair halves; out = (outT/r).T.
"""

import sys

sys.path.insert(0, "/opt/trn_rl_repo")

import numpy as np
import ml_dtypes

B, S, DM, DQ = 4, 4096, 256, 64
T = [32, 24, 16, 8]  # padded template: key-blocks per slot
STRIPES_A = [7, 5, 2, 0]  # R_A = [32, 24, 12, 4]
STRIPES_B = [6, 4, 3, 1]  # R_B = [28, 20, 16, 8]
# Per-pair mask thresholds for the last 4 pairs of a slot (f16-safe:
# QK values lie in [-255, 511], so +-1000 mean all-ones / all-zeros).
# exact slot (R == T): pairs are [ones, ones, tri(0/128), tri(256/384)]
# padded slot (R == T-4): pairs are [tri(0/128), tri(256/384), zero, zero]
TH_EXACT = [-1000.0, -1000.0, 0.0, 256.0]
TH_PAD = [0.0, 256.0, 1000.0, 1000.0]

_CACHE = {}


def _build_nc():
    import concourse.bass as bass  # noqa: F401
    import concourse.tile as tile
    from concourse import bacc, mybir

    dt = mybir.dt
    f32, bf, f16 = dt.float32, dt.bfloat16, dt.float16

    nc = bacc.Bacc(
        "TRN2",
        target_bir_lowering=False,
        debug=False,
        enable_asserts=False,
        num_devices=8,
    )

    def din(name, shape, d):
        return nc.dram_tensor(name, shape, d, kind="ExternalInput").ap()

    eq = din("eq", [256, 2048], bf)
    ek = din("ek", [256, 4096], bf)  # column-reordered: even blocks, then odd
    ev = din("ev", [256, 4096], bf)
    # all projection weights in one tensor (one DMA per 128-partition half):
    # cols 0:64 Wk.T, 64:192 Wq.T duplicated, 192:448 Wv.T
    wkqv = din("wkqv", [256, 448], bf)
    # thresholds + QK iota in one f16 tensor: cols 0:16 th, 16:1040 QK
    # (QK[p, i2*512+qi] = qi-128*i2-p; th in {-1000,0,256,1000})
    thqk = din("thqk", [128, 1040], f16)
    outT = nc.dram_tensor("outT", [256, 2048], bf, kind="ExternalOutput").ap()
    acc_out = nc.dram_tensor("acc", [128, 4096], f16, kind="ExternalOutput").ap()

    Exp = mybir.ActivationFunctionType.Exp
    GE = mybir.AluOpType.is_ge

    with tile.TileContext(nc) as tc:
        from contextlib import ExitStack

        with ExitStack() as ctx:
            const = ctx.enter_context(tc.tile_pool(name="const", bufs=1))

            # ---- persistent SBUF tensors ----
            eq_sb = [const.tile([128, 2048], bf, tag=f"eq{h}", name=f"eq{h}") for h in range(2)]
            ek_sb = [const.tile([128, 4096], bf, tag=f"ek{h}", name=f"ek{h}") for h in range(2)]
            ev_sb = [const.tile([128, 4096], bf, tag=f"ev{h}", name=f"ev{h}") for h in range(2)]
            wkqv_sb = const.tile([128, 896], bf, tag="wkqv", name="wkqv")
            # per-half weight views into wkqv_sb
            wk_h = [wkqv_sb[:, h * 448 : h * 448 + 64] for h in range(2)]
            wq_h = [wkqv_sb[:, h * 448 + 64 : h * 448 + 192] for h in range(2)]
            wv_h = [wkqv_sb[:, h * 448 + 192 : h * 448 + 448] for h in range(2)]
            thqk_sb = const.tile([128, 1040], f16, tag="thqk", name="thqk")
            qk = thqk_sb[:, 16:1040]
            th_sb = const.tile([128, 16], f32, tag="th32", name="th32")
            qT = const.tile([128, 2048], bf, tag="qT", name="qT")  # dup halves
            kTp = const.tile([128, 2048], bf, tag="kTp", name="kTp")  # pair-packed
            v_sb = const.tile([128, 32 * 256], bf, tag="v", name="v")
            acc = const.tile([128, 4096], f16, tag="acc", name="acc")
            mk = const.tile([128, 16 * 1024], bf, tag="mk", name="mk")

            # Input DMAs are issued from both SP and Activation queues (half
            # each) and staged per phase: phases 0-1 up front, later phases
            # from inside the pipeline so issue time hides under compute.
            # Phase-0 order is criticality-driven: th/qk (mask gen), wk/wq,
            # ek (kTp proj), eq (qT proj), wv, ev.
            def dma_phase(p, j):
                # phases 0-1 are issued up front across FOUR queues (queue
                # issue+transfer serialize at ~1us per 128KB on one queue);
                # later phases go all-SP so ACT/DVE stay free for compute.
                cs_q = slice(j * 512, (j + 1) * 512)
                if p == 0:
                    nc.scalar.dma_start(thqk_sb[:], thqk[:])
                for reg in range(2):  # 0: even region, 1: odd region
                    cs = slice(reg * 2048 + p * 512, reg * 2048 + (p + 1) * 512)
                    if p == 0:
                        nc.sync.dma_start(ek_sb[0][:, cs], ek[0:128, cs])
                        nc.gpsimd.dma_start(ek_sb[1][:, cs], ek[128:256, cs])
                    elif p == 1:
                        nc.gpsimd.dma_start(ek_sb[0][:, cs], ek[0:128, cs])
                        nc.gpsimd.dma_start(ek_sb[1][:, cs], ek[128:256, cs])
                    else:
                        nc.sync.dma_start(ek_sb[0][:, cs], ek[0:128, cs])
                        nc.sync.dma_start(ek_sb[1][:, cs], ek[128:256, cs])
                if p == 0:
                    nc.sync.dma_start(wkqv_sb[:, 0:448], wkqv[0:128, :])
                    nc.scalar.dma_start(wkqv_sb[:, 448:896], wkqv[128:256, :])
                alt = nc.scalar if p == 0 else nc.sync
                nc.sync.dma_start(eq_sb[0][:, cs_q], eq[0:128, cs_q])
                alt.dma_start(eq_sb[1][:, cs_q], eq[128:256, cs_q])
                for cc in EV_CH[p]:
                    cs = slice(cc * 512, (cc + 1) * 512)
                    nc.sync.dma_start(ev_sb[0][:, cs], ev[0:128, cs])
                    alt.dma_start(ev_sb[1][:, cs], ev[128:256, cs])

            PHASES = ((0, 3), (1, 2), (2, 1), (3, 0))
            V_QUOTA = ((0, 1, 2, 3, 4, 5), (6, 7, 8, 9), (10, 11, 12, 13), (14, 15))
            EV_CH = ((0, 1, 2), (3, 4), (5, 6), (7,))
            # scr memset first: it must sit ahead of the phase-1 ek DMAs in
            # the GpSimd FIFO or the PE warm-up stalls behind 4 transfers
            scr = const.tile([128, 512], bf, tag="scr", name="scr")
            nc.gpsimd.memset(scr[:], 0.0)
            dma_phase(0, 3)
            dma_phase(1, 2)

            def mask_gen(j):
                # mk[t] = (QK >= th[t]) in bf16, on DVE (~478ns each; GpSimd's
                # tensor_scalar ucode is 30x slower for this op, do NOT use it)
                for m in range(4):
                    t = j * 4 + m
                    nc.vector.tensor_scalar(
                        mk[:, t * 1024 : (t + 1) * 1024], qk[:], th_sb[:, t : t + 1], None, GE
                    )

            # Slot-3 masks up front (its first score groups are all masked);
            # slot j+1's masks are generated during phase p's tail so the
            # 478ns-each DVE ops never sit ahead of critical qT/kTp casts.
            nc.vector.tensor_copy(th_sb[:], thqk_sb[:, 0:16])  # f16 -> f32
            mask_gen(3)

            # PSUM: psc 2x[128,1024] (4 banks) + 4 po banks = 8; the proj
            # tiles share the psc pool (use cols 0:512 of a [128,1024] tile)
            psc = ctx.enter_context(tc.tile_pool(name="psc", bufs=2, space="PSUM"))
            po_pool = ctx.enter_context(tc.tile_pool(name="po", bufs=1, space="PSUM"))
            epool = ctx.enter_context(tc.tile_pool(name="e", bufs=8))
            opool = ctx.enter_context(tc.tile_pool(name="o", bufs=2))

            def pp_tile():
                return psc.tile([128, 1024], f32, tag="ps", name="ps")

            for _ in range(6):
                ps = pp_tile()
                nc.tensor.matmul(
                    ps[:, 0:512], scr[:, 0:128], scr[:], start=True, stop=True
                )

            def proj_qk(p, j):
                # kTp chunk p: even blocks -> partitions 0:64, odd -> 64:128
                # (the two 64-partition outputs col-tile and run concurrent)
                ps = pp_tile()
                for half in range(2):
                    dst = ps[half * 64 : (half + 1) * 64, 0:512]
                    for h in range(2):
                        nc.tensor.matmul(
                            dst,
                            wk_h[h],
                            ek_sb[h][:, half * 2048 + p * 512 : half * 2048 + (p + 1) * 512],
                            start=(h == 0),
                            stop=(h == 1),
                        )
                nc.vector.tensor_copy(kTp[:, p * 512 : (p + 1) * 512], ps[:, 0:512])
                # qT chunk j (duplicated onto both halves by the dup'd wq)
                ps = pp_tile()
                for h in range(2):
                    nc.tensor.matmul(
                        ps[:, 0:512],
                        wq_h[h],
                        eq_sb[h][:, j * 512 : (j + 1) * 512],
                        start=(h == 0),
                        stop=(h == 1),
                    )
                nc.vector.tensor_copy(qT[:, j * 512 : (j + 1) * 512], ps[:, 0:512])

            def v_proj(tiles):
                # v pairs (natural [keys, 256] bf16); copies go 2-of-3 on
                # DVE, 1-of-3 on ACT so ACT keeps the exp cadence
                for n, i in enumerate(tiles):
                    ps = pp_tile()
                    for s in range(2):
                        t = 2 * i + s
                        for h in range(2):
                            nc.tensor.matmul(
                                ps[:, s * 256 : (s + 1) * 256],
                                ev_sb[h][:, t * 128 : (t + 1) * 128],
                                wv_h[h],
                                start=(h == 0),
                                stop=(h == 1),
                            )
                    dst = v_sb[:, i * 512 : (i + 1) * 512]
                    # 1-of-3 on ACT: more ACT copies break the back-to-back
                    # exp pipelining, more DVE copies choke the acc chain
                    if n % 3 == 2:
                        nc.scalar.copy(dst, ps[:, 0:512])
                    else:
                        nc.vector.tensor_copy(dst, ps[:, 0:512])

            G, GLAG = 2, 2  # score-pair group size, pv lag in groups

            class Slot:
                def __init__(self, p, j):
                    self.p, self.j = p, j
                    self.npairs = T[j] // 2
                    self.ngroups = self.npairs // G
                    self.po0a = po_pool.tile([128, 512], f32, tag="po0a", name="po0a")
                    self.po0b = po_pool.tile([128, 512], f32, tag="po0b", name="po0b")
                    self.po1a = po_pool.tile([128, 512], f32, tag="po1a", name="po1a")
                    self.po1b = po_pool.tile([128, 512], f32, tag="po1b", name="po1b")
                    self.qs_top = qT[0:64, j * 512 : (j + 1) * 512]
                    self.qs_bot = qT[64:128, j * 512 : (j + 1) * 512]
                    self.acc_j = acc[:, j * 1024 : (j + 1) * 1024]
                    self.es = [None] * self.npairs
                    # Process masked pairs right after the pipeline fills
                    # (their DVE mask-muls get full lag slack) and end every
                    # slot on unmasked pairs so the drain has no DVE dep.
                    np_ = self.npairs
                    if np_ > 4:
                        k = min(4, np_ - 8)
                        self.proc = (
                            list(range(k)) + list(range(np_ - 4, np_)) + list(range(k, np_ - 4))
                        )
                    else:
                        self.proc = list(range(np_))

                def pv_group(self, h):
                    # pv split into 64-row sub-matmuls: keys 0:64 stream on
                    # PE rows 0:64 (pipe A -> po0a/po1a), keys 64:128 on rows
                    # 64:128 (pipe B -> po0b/po1b). The 64-row score matmuls
                    # share the same pipes, so there is NO 64<->128-row
                    # stationary-switch drain stall. Each PSUM bank has
                    # exactly ONE writer pipe (the shared-bank interleave
                    # variant hung the device - never share a bank).
                    for dd in range(h * G, h * G + G):
                        e = self.es[dd]
                        for half in range(2):
                            u = 2 * self.proc[dd] + half
                            first = dd == 0 and half == 0
                            last = dd == self.npairs - 1 and half == 1
                            c0 = slice(u * 256, u * 256 + 128)
                            c1 = slice(u * 256 + 128, (u + 1) * 256)
                            e0 = e[0:64, half * 512 : (half + 1) * 512]
                            e1 = e[64:128, half * 512 : (half + 1) * 512]
                            nc.tensor.matmul(
                                self.po0a[:], v_sb[0:64, c0], e0,
                                start=first, stop=last,
                            )
                            nc.tensor.matmul(
                                self.po0b[:], v_sb[64:128, c0], e1,
                                start=first, stop=last,
                            )
                            nc.tensor.matmul(
                                self.po1a[:], v_sb[0:64, c1], e0,
                                start=first, stop=last,
                            )
                            nc.tensor.matmul(
                                self.po1b[:], v_sb[64:128, c1], e1,
                                start=first, stop=last,
                            )

                def sc_group(self, g):
                    # scores for the G pairs of group g, then exp/mask/acc
                    for d in range(g * G, g * G + G):
                        tp = self.proc[d]
                        ps = psc.tile([128, 1024], f32, tag="ps", name="ps")
                        nc.tensor.matmul(
                            ps[:, 0:512],
                            kTp[0:64, tp * 128 : (tp + 1) * 128],
                            self.qs_top,
                            start=True,
                            stop=True,
                        )
                        nc.tensor.matmul(
                            ps[:, 512:1024],
                            kTp[64:128, tp * 128 : (tp + 1) * 128],
                            self.qs_bot,
                            start=True,
                            stop=True,
                        )
                        e = epool.tile([128, 1024], bf, tag="e", name="e")
                        nc.scalar.activation(e[:], ps[:], Exp, scale=0.125)
                        if tp >= self.npairs - 4:
                            t = self.j * 4 + tp - (self.npairs - 4)
                            nc.vector.tensor_mul(
                                e[:], e[:], mk[:, t * 1024 : (t + 1) * 1024]
                            )
                        if d == 0:
                            nc.vector.tensor_copy(self.acc_j, e[:])
                        else:
                            nc.vector.tensor_add(self.acc_j, self.acc_j, e[:])
                        self.es[d] = e

                def fin(self, last_phase):
                    nc.sync.dma_start(
                        acc_out[:, self.j * 1024 : (self.j + 1) * 1024], self.acc_j
                    )
                    o0 = opool.tile([128, 512], bf, tag="o0", name="o0")
                    o1 = opool.tile([128, 512], bf, tag="o1", name="o1")
                    t0 = opool.tile([128, 512], bf, tag="t0", name="t0")
                    t1 = opool.tile([128, 512], bf, tag="t1", name="t1")
                    # TensorTensor cannot read two PSUM operands: stage the
                    # B-pipe banks to SBUF on ACT (idle at the boundary),
                    # then one-PSUM-operand adds on DVE.
                    nc.scalar.copy(t0[:], self.po0b[:])
                    nc.scalar.copy(t1[:], self.po1b[:])
                    nc.vector.tensor_add(o0[:], self.po0a[:], t0[:])
                    nc.vector.tensor_add(o1[:], self.po1a[:], t1[:])
                    j = self.j
                    nc.sync.dma_start(outT[0:128, j * 512 : (j + 1) * 512], o0[:])
                    if last_phase:  # final slot: second out DMA from ACT queue
                        nc.scalar.dma_start(outT[128:256, j * 512 : (j + 1) * 512], o1[:])
                    else:
                        nc.sync.dma_start(outT[128:256, j * 512 : (j + 1) * 512], o1[:])

            # ---- software-pipelined emission across phases ----
            # Per phase p (slot j): [sc0, prev.pv(ng-1), sc1, prev.fin,
            # v_proj(p), {pv(g-2), sc(g)}..., proj(p+1), mask_gen(j+1),
            # pv(ng-2)]; the final pv drain and output copies of slot j land
            # inside phase p+1's head, overlapped with its first exps.
            proj_qk(0, 3)
            prev = None
            for p, j in PHASES:
                s = Slot(p, j)
                s.sc_group(0)
                if prev is not None:
                    prev.pv_group(prev.ngroups - 1)
                s.sc_group(1)
                if prev is not None:
                    prev.fin(last_phase=False)
                # phase 0: only tiles 0-1 are needed before pv(0); tiles 2-3
                # are emitted in the tail so the first exps start sooner
                v_proj(V_QUOTA[p][:2] if p == 0 else V_QUOTA[p])
                for g in range(2, s.ngroups):
                    s.pv_group(g - 2)
                    s.sc_group(g)
                if p < 3:
                    proj_qk(p + 1, PHASES[p + 1][1])
                    mask_gen(PHASES[p + 1][1])
                if p == 0:
                    v_proj(V_QUOTA[0][2:])
                s.pv_group(s.ngroups - 2)
                if p + 2 <= 3:
                    dma_phase(*PHASES[p + 2])
                prev = s
            prev.pv_group(prev.ngroups - 1)
            prev.fin(last_phase=True)

    nc.compile()
    return nc


def kernel(encodings_for_q, encodings_for_k, encodings_for_v, mask, Wq, Wk, Wv):
    from concourse.bass_utils import run_bass_kernel_spmd

    if "nc" not in _CACHE:
        _CACHE["nc"] = _build_nc()
    nc = _CACHE["nc"]

    bf = ml_dtypes.bfloat16
    # [Wk.T | Wq.T dup | Wv.T] -> [256, 448]
    wkqv_host = np.ascontiguousarray(
        np.concatenate([Wk.T, Wq.T, Wq.T, Wv.T], axis=1), dtype=bf
    )
    # QK[p, i2*512 + qi] = qi - 128*i2 - p
    qi = np.arange(512, dtype=np.float32)
    i2 = np.arange(2, dtype=np.float32)
    pch = np.arange(128, dtype=np.float32)
    qk_host = (
        (qi[None, None, :] - 128 * i2[None, :, None] - pch[:, None, None])
        .reshape(128, 1024)
        .astype(np.float16)
    )

    in_maps = []
    metas = []
    for c in range(8):
        b, t = c // 2, c % 2
        stripes = STRIPES_A if t == 0 else STRIPES_B
        eqT = np.concatenate(
            [encodings_for_q[b, st * 512 : (st + 1) * 512, :].T for st in stripes],
            axis=1,
        )
        ekT = encodings_for_k[b].T.reshape(256, 32, 128)
        ek_reord = np.concatenate([ekT[:, 0::2, :], ekT[:, 1::2, :]], axis=1).reshape(
            256, 4096
        )
        # thresholds: slot j exact if R[j] == T[j]
        thr = np.empty((16,), dtype=np.float16)
        for j in range(4):
            R = 4 * (stripes[j] + 1)
            vals = TH_EXACT if R == T[j] else TH_PAD
            thr[j * 4 : (j + 1) * 4] = vals
        thqk_host = np.ascontiguousarray(
            np.concatenate([np.broadcast_to(thr, (128, 16)), qk_host], axis=1),
            dtype=np.float16,
        )
        in_maps.append(
            {
                "eq": np.ascontiguousarray(eqT, dtype=bf),
                "ek": np.ascontiguousarray(ek_reord, dtype=bf),
                "ev": np.ascontiguousarray(encodings_for_v[b].T, dtype=bf),
                "wkqv": wkqv_host,
                "thqk": thqk_host,
            }
        )
        metas.append((b, stripes))

    res = run_bass_kernel_spmd(nc, in_maps, core_ids=list(range(8)))
    _CACHE["last_res"] = res

    out = np.empty((B, S, DM), dtype=np.float32)
    for c in range(8):
        b, stripes = metas[c]
        oT = res.results[c]["outT"].astype(np.float32)
        a = res.results[c]["acc"].astype(np.float32)
        for j, st in enumerate(stripes):
            r = a[:, j * 1024 : j * 1024 + 512].sum(0) + a[
                :, j * 1024 + 512 : (j + 1) * 1024
            ].sum(0)
            blk = oT[:, j * 512 : (j + 1) * 512] / r[None, :]
            out[b, st * 512 : (st + 1) * 512, :] = blk.T
    return out


# revision 43
# speedup vs baseline: 1.1534x; 1.1500x over previous
"""Causal attention head on 8 trn2 NeuronCores.

Sharding: core c = (batch b = c//2, type t = c%2). Each core handles 4
query stripes of 512 of its batch. Causal balance: type A gets stripes
[7,5,2,0] with real key-block counts R_A=[32,24,12,4]; type B stripes
[6,4,3,1] with R_B=[28,20,16,8]. One SPMD program: every core runs the
padded template T=[32,24,16,8]; per-core behaviour comes only from input
data (per-core threshold scalars select ones/triangle/zero mask tiles).

Everything on the PE array is bf16. Score matmuls pack two key-blocks
per issue via PE row tiling: kT pairs live on partition halves 0:64 /
64:128 (host interleaves ek into even/odd block regions), qT is
duplicated onto both halves by a column-duplicated Wq.

Attention processes pairs (2 key-blocks) in GROUPS of 2 to amortize the
PE stationary-switch stall (64-row score tiles vs 128-row pv tiles cost
~106ns per transition: drain + non-overlapped LDWEIGHTS). Per group g:
pv matmuls for group g-2 (8 MMs), then scores for the 2 pairs of group
g (psc double-buffer holds exactly 2 pairs), exp per pair on ACT, mask
mul (DVE) on the last-4 pairs, acc += e (DVE fp16).

The emission is software-pipelined ACROSS slots so the PE never idles at
a slot boundary: after slot j's last score group we emit [proj(p+1),
mask-gen(slot j+1), pv(ng-2)], and slot j+1's first two score groups
(which have no pv partner: GLAG=2) interleave with slot j's final pv
drain group and output copies. Input DMAs are split across the SP and
ACT issue queues with the critical slot-3 set (th/qk/wk/wq/ek/eq)
first; dummy matmuls keep the PE busy under the DMA wait so the HAM
clock-gate opens (1.2->2.4 GHz) early. QK (query-index minus key-index
iota) comes in as a host input so slot-3 masks only wait on two small
DMAs. Outputs: outT bf16 [256,2048] (unnormalized; o0 drained via DVE,
o1 via ACT in parallel), acc fp16 [128,4096]. Host: r = colsum(acc)
folded over pair halves; out = (outT/r).T.
"""

import sys

sys.path.insert(0, "/opt/trn_rl_repo")

import numpy as np
import ml_dtypes

B, S, DM, DQ = 4, 4096, 256, 64
T = [32, 24, 16, 8]  # padded template: key-blocks per slot
STRIPES_A = [7, 5, 2, 0]  # R_A = [32, 24, 12, 4]
STRIPES_B = [6, 4, 3, 1]  # R_B = [28, 20, 16, 8]
# Per-pair mask thresholds for the last 4 pairs of a slot (f16-safe:
# QK values lie in [-255, 511], so +-1000 mean all-ones / all-zeros).
# exact slot (R == T): pairs are [ones, ones, tri(0/128), tri(256/384)]
# padded slot (R == T-4): pairs are [tri(0/128), tri(256/384), zero, zero]
TH_EXACT = [-1000.0, -1000.0, 0.0, 256.0]
TH_PAD = [0.0, 256.0, 1000.0, 1000.0]

_CACHE = {}


def _build_nc():
    import concourse.bass as bass  # noqa: F401
    import concourse.tile as tile
    from concourse import bacc, mybir

    dt = mybir.dt
    f32, bf, f16 = dt.float32, dt.bfloat16, dt.float16

    nc = bacc.Bacc(
        "TRN2",
        target_bir_lowering=False,
        debug=False,
        enable_asserts=False,
        num_devices=8,
    )

    def din(name, shape, d):
        return nc.dram_tensor(name, shape, d, kind="ExternalInput").ap()

    eq = din("eq", [256, 2048], bf)
    ek = din("ek", [256, 4096], bf)  # column-reordered: even blocks, then odd
    ev = din("ev", [256, 4096], bf)
    # all projection weights in one tensor (one DMA per 128-partition half):
    # cols 0:64 Wk.T, 64:192 Wq.T duplicated, 192:448 Wv.T
    wkqv = din("wkqv", [256, 448], bf)
    # thresholds + QK iota in one f16 tensor: cols 0:16 th, 16:1040 QK
    # (QK[p, i2*512+qi] = qi-128*i2-p; th in {-1000,0,256,1000})
    thqk = din("thqk", [128, 1040], f16)
    outT = nc.dram_tensor("outT", [256, 2048], bf, kind="ExternalOutput").ap()
    acc_out = nc.dram_tensor("acc", [128, 4096], f16, kind="ExternalOutput").ap()

    Exp = mybir.ActivationFunctionType.Exp
    GE = mybir.AluOpType.is_ge

    with tile.TileContext(nc) as tc:
        from contextlib import ExitStack

        with ExitStack() as ctx:
            const = ctx.enter_context(tc.tile_pool(name="const", bufs=1))

            # ---- persistent SBUF tensors ----
            eq_sb = [const.tile([128, 2048], bf, tag=f"eq{h}", name=f"eq{h}") for h in range(2)]
            ek_sb = [const.tile([128, 4096], bf, tag=f"ek{h}", name=f"ek{h}") for h in range(2)]
            ev_sb = [const.tile([128, 4096], bf, tag=f"ev{h}", name=f"ev{h}") for h in range(2)]
            wkqv_sb = const.tile([128, 896], bf, tag="wkqv", name="wkqv")
            # per-half weight views into wkqv_sb
            wk_h = [wkqv_sb[:, h * 448 : h * 448 + 64] for h in range(2)]
            wq_h = [wkqv_sb[:, h * 448 + 64 : h * 448 + 192] for h in range(2)]
            wv_h = [wkqv_sb[:, h * 448 + 192 : h * 448 + 448] for h in range(2)]
            thqk_sb = const.tile([128, 1040], f16, tag="thqk", name="thqk")
            qk = thqk_sb[:, 16:1040]
            th_sb = const.tile([128, 16], f32, tag="th32", name="th32")
            qT = const.tile([128, 2048], bf, tag="qT", name="qT")  # dup halves
            kTp = const.tile([128, 2048], bf, tag="kTp", name="kTp")  # pair-packed
            v_sb = const.tile([128, 32 * 256], bf, tag="v", name="v")
            acc = const.tile([128, 4096], f16, tag="acc", name="acc")
            mk = const.tile([128, 16 * 1024], bf, tag="mk", name="mk")

            # Input DMAs are issued from both SP and Activation queues (half
            # each) and staged per phase: phases 0-1 up front, later phases
            # from inside the pipeline so issue time hides under compute.
            # Phase-0 order is criticality-driven: th/qk (mask gen), wk/wq,
            # ek (kTp proj), eq (qT proj), wv, ev.
            def dma_phase(p, j):
                # phases 0-1 are issued up front across FOUR queues (queue
                # issue+transfer serialize at ~1us per 128KB on one queue);
                # later phases go all-SP so ACT/DVE stay free for compute.
                cs_q = slice(j * 512, (j + 1) * 512)
                if p == 0:
                    nc.scalar.dma_start(thqk_sb[:], thqk[:])
                for reg in range(2):  # 0: even region, 1: odd region
                    cs = slice(reg * 2048 + p * 512, reg * 2048 + (p + 1) * 512)
                    if p == 0:
                        nc.sync.dma_start(ek_sb[0][:, cs], ek[0:128, cs])
                        nc.gpsimd.dma_start(ek_sb[1][:, cs], ek[128:256, cs])
                    elif p == 1:
                        nc.gpsimd.dma_start(ek_sb[0][:, cs], ek[0:128, cs])
                        nc.gpsimd.dma_start(ek_sb[1][:, cs], ek[128:256, cs])
                    else:
                        nc.sync.dma_start(ek_sb[0][:, cs], ek[0:128, cs])
                        nc.sync.dma_start(ek_sb[1][:, cs], ek[128:256, cs])
                if p == 0:
                    nc.sync.dma_start(wkqv_sb[:, 0:448], wkqv[0:128, :])
                    nc.scalar.dma_start(wkqv_sb[:, 448:896], wkqv[128:256, :])
                alt = nc.scalar if p == 0 else nc.sync
                nc.sync.dma_start(eq_sb[0][:, cs_q], eq[0:128, cs_q])
                alt.dma_start(eq_sb[1][:, cs_q], eq[128:256, cs_q])
                for cc in EV_CH[p]:
                    cs = slice(cc * 512, (cc + 1) * 512)
                    nc.sync.dma_start(ev_sb[0][:, cs], ev[0:128, cs])
                    alt.dma_start(ev_sb[1][:, cs], ev[128:256, cs])

            PHASES = ((0, 3), (1, 2), (2, 1), (3, 0))
            V_QUOTA = ((0, 1, 2, 3, 4, 5), (6, 7, 8, 9), (10, 11, 12, 13), (14, 15))
            EV_CH = ((0, 1, 2), (3, 4), (5, 6), (7,))
            # scr memset first: it must sit ahead of the phase-1 ek DMAs in
            # the GpSimd FIFO or the PE warm-up stalls behind 4 transfers
            scr = const.tile([128, 512], bf, tag="scr", name="scr")
            nc.gpsimd.memset(scr[:], 0.0)
            dma_phase(0, 3)
            dma_phase(1, 2)

            def mask_gen(j):
                # mk[t] = (QK >= th[t]) in bf16, on DVE (~478ns each; GpSimd's
                # tensor_scalar ucode is 30x slower for this op, do NOT use it)
                for m in range(4):
                    t = j * 4 + m
                    nc.vector.tensor_scalar(
                        mk[:, t * 1024 : (t + 1) * 1024], qk[:], th_sb[:, t : t + 1], None, GE
                    )

            # Slot-3 masks up front (its first score groups are all masked);
            # slot j+1's masks are generated during phase p's tail so the
            # 478ns-each DVE ops never sit ahead of critical qT/kTp casts.
            nc.vector.tensor_copy(th_sb[:], thqk_sb[:, 0:16])  # f16 -> f32
            mask_gen(3)

            pp = ctx.enter_context(tc.tile_pool(name="pp", bufs=2, space="PSUM"))
            psc = ctx.enter_context(tc.tile_pool(name="psc", bufs=2, space="PSUM"))
            po_pool = ctx.enter_context(tc.tile_pool(name="po", bufs=1, space="PSUM"))
            epool = ctx.enter_context(tc.tile_pool(name="e", bufs=8))
            opool = ctx.enter_context(tc.tile_pool(name="o", bufs=2))

            def pp_tile():
                return pp.tile([128, 512], f32, tag="ps", name="ps")

            for _ in range(6):
                ps = pp_tile()
                nc.tensor.matmul(ps[:], scr[:, 0:128], scr[:], start=True, stop=True)

            def proj_qk(p, j):
                # kTp chunk p: even blocks -> partitions 0:64, odd -> 64:128
                # (the two 64-partition outputs col-tile and run concurrent)
                ps = pp_tile()
                for half in range(2):
                    dst = ps[half * 64 : (half + 1) * 64, :]
                    for h in range(2):
                        nc.tensor.matmul(
                            dst,
                            wk_h[h],
                            ek_sb[h][:, half * 2048 + p * 512 : half * 2048 + (p + 1) * 512],
                            start=(h == 0),
                            stop=(h == 1),
                        )
                nc.vector.tensor_copy(kTp[:, p * 512 : (p + 1) * 512], ps[:])
                # qT chunk j (duplicated onto both halves by the dup'd wq)
                ps = pp_tile()
                for h in range(2):
                    nc.tensor.matmul(
                        ps[:],
                        wq_h[h],
                        eq_sb[h][:, j * 512 : (j + 1) * 512],
                        start=(h == 0),
                        stop=(h == 1),
                    )
                nc.vector.tensor_copy(qT[:, j * 512 : (j + 1) * 512], ps[:])

            def v_proj(tiles):
                # v pairs (natural [keys, 256] bf16); copies go 2-of-3 on
                # DVE, 1-of-3 on ACT so ACT keeps the exp cadence
                for n, i in enumerate(tiles):
                    ps = pp_tile()
                    for s in range(2):
                        t = 2 * i + s
                        for h in range(2):
                            nc.tensor.matmul(
                                ps[:, s * 256 : (s + 1) * 256],
                                ev_sb[h][:, t * 128 : (t + 1) * 128],
                                wv_h[h],
                                start=(h == 0),
                                stop=(h == 1),
                            )
                    dst = v_sb[:, i * 512 : (i + 1) * 512]
                    # 1-of-3 on ACT: more ACT copies break the back-to-back
                    # exp pipelining, more DVE copies choke the acc chain
                    if n % 3 == 2:
                        nc.scalar.copy(dst, ps[:])
                    else:
                        nc.vector.tensor_copy(dst, ps[:])

            G, GLAG = 2, 2  # score-pair group size, pv lag in groups

            class Slot:
                def __init__(self, p, j):
                    self.p, self.j = p, j
                    self.npairs = T[j] // 2
                    self.ngroups = self.npairs // G
                    self.po0 = po_pool.tile([128, 512], f32, tag="po0", name="po0")
                    self.po1 = po_pool.tile([128, 512], f32, tag="po1", name="po1")
                    self.qs_top = qT[0:64, j * 512 : (j + 1) * 512]
                    self.qs_bot = qT[64:128, j * 512 : (j + 1) * 512]
                    self.acc_j = acc[:, j * 1024 : (j + 1) * 1024]
                    self.es = [None] * self.npairs
                    # Process masked pairs right after the pipeline fills
                    # (their DVE mask-muls get full lag slack) and end every
                    # slot on unmasked pairs so the drain has no DVE dep.
                    np_ = self.npairs
                    if np_ > 4:
                        k = min(4, np_ - 8)
                        self.proc = (
                            list(range(k)) + list(range(np_ - 4, np_)) + list(range(k, np_ - 4))
                        )
                    else:
                        self.proc = list(range(np_))

                def pv_group(self, h):
                    # 4 pv matmuls per pair, 8 per group; LDWEIGHTS chains
                    # through the background weight buffer at 216ns cadence.
                    # (64-row pv splits were tried twice: shared-bank
                    # interleave hangs the device; per-pipe banks work and
                    # reach ~1070ns/pair steady BUT need the pp pool's 2
                    # PSUM banks, and proj tiles rotating through psc
                    # serialize slot boundaries for a net loss. Keep 128-row.)
                    for dd in range(h * G, h * G + G):
                        e = self.es[dd]
                        for half in range(2):
                            u = 2 * self.proc[dd] + half
                            eh = e[:, half * 512 : (half + 1) * 512]
                            first = dd == 0 and half == 0
                            last = dd == self.npairs - 1 and half == 1
                            nc.tensor.matmul(
                                self.po0[:],
                                v_sb[:, u * 256 : u * 256 + 128],
                                eh,
                                start=first,
                                stop=last,
                            )
                            nc.tensor.matmul(
                                self.po1[:],
                                v_sb[:, u * 256 + 128 : (u + 1) * 256],
                                eh,
                                start=first,
                                stop=last,
                            )

                def sc_group(self, g):
                    # scores for the G pairs of group g, then exp/mask/acc
                    for d in range(g * G, g * G + G):
                        tp = self.proc[d]
                        ps = psc.tile([128, 1024], f32, tag="ps", name="ps")
                        nc.tensor.matmul(
                            ps[:, 0:512],
                            kTp[0:64, tp * 128 : (tp + 1) * 128],
                            self.qs_top,
                            start=True,
                            stop=True,
                        )
                        nc.tensor.matmul(
                            ps[:, 512:1024],
                            kTp[64:128, tp * 128 : (tp + 1) * 128],
                            self.qs_bot,
                            start=True,
                            stop=True,
                        )
                        e = epool.tile([128, 1024], bf, tag="e", name="e")
                        nc.scalar.activation(e[:], ps[:], Exp, scale=0.125)
                        if tp >= self.npairs - 4:
                            t = self.j * 4 + tp - (self.npairs - 4)
                            nc.vector.tensor_mul(
                                e[:], e[:], mk[:, t * 1024 : (t + 1) * 1024]
                            )
                        if d == 0:
                            nc.vector.tensor_copy(self.acc_j, e[:])
                        else:
                            nc.vector.tensor_add(self.acc_j, self.acc_j, e[:])
                        self.es[d] = e

                def fin(self, last_phase):
                    nc.sync.dma_start(
                        acc_out[:, self.j * 1024 : (self.j + 1) * 1024], self.acc_j
                    )
                    o0 = opool.tile([128, 512], bf, tag="o0", name="o0")
                    o1 = opool.tile([128, 512], bf, tag="o1", name="o1")
                    # o0 via DVE, o1 via ACT: the drains run in parallel and
                    # ACT never blocks long behind the pv-drain semaphore.
                    nc.vector.tensor_copy(o0[:], self.po0[:])
                    nc.scalar.copy(o1[:], self.po1[:])
                    j = self.j
                    nc.sync.dma_start(outT[0:128, j * 512 : (j + 1) * 512], o0[:])
                    if last_phase:  # final slot: second out DMA from ACT queue
                        nc.scalar.dma_start(outT[128:256, j * 512 : (j + 1) * 512], o1[:])
                    else:
                        nc.sync.dma_start(outT[128:256, j * 512 : (j + 1) * 512], o1[:])

            # ---- software-pipelined emission across phases ----
            # Per phase p (slot j): [sc0, prev.pv(ng-1), sc1, prev.fin,
            # v_proj(p), {pv(g-2), sc(g)}..., proj(p+1), mask_gen(j+1),
            # pv(ng-2)]; the final pv drain and output copies of slot j land
            # inside phase p+1's head, overlapped with its first exps.
            proj_qk(0, 3)
            prev = None
            for p, j in PHASES:
                s = Slot(p, j)
                s.sc_group(0)
                if prev is not None:
                    prev.pv_group(prev.ngroups - 1)
                s.sc_group(1)
                if prev is not None:
                    prev.fin(last_phase=False)
                # phase 0: only tiles 0-1 are needed before pv(0); tiles 2-3
                # are emitted in the tail so the first exps start sooner
                v_proj(V_QUOTA[p][:2] if p == 0 else V_QUOTA[p])
                for g in range(2, s.ngroups):
                    s.pv_group(g - 2)
                    s.sc_group(g)
                if p < 3:
                    proj_qk(p + 1, PHASES[p + 1][1])
                    mask_gen(PHASES[p + 1][1])
                if p == 0:
                    v_proj(V_QUOTA[0][2:])
                s.pv_group(s.ngroups - 2)
                if p + 2 <= 3:
                    dma_phase(*PHASES[p + 2])
                prev = s
            prev.pv_group(prev.ngroups - 1)
            prev.fin(last_phase=True)

    nc.compile()
    return nc


def kernel(encodings_for_q, encodings_for_k, encodings_for_v, mask, Wq, Wk, Wv):
    from concourse.bass_utils import run_bass_kernel_spmd

    if "nc" not in _CACHE:
        _CACHE["nc"] = _build_nc()
    nc = _CACHE["nc"]

    bf = ml_dtypes.bfloat16
    # [Wk.T | Wq.T dup | Wv.T] -> [256, 448]
    wkqv_host = np.ascontiguousarray(
        np.concatenate([Wk.T, Wq.T, Wq.T, Wv.T], axis=1), dtype=bf
    )
    # QK[p, i2*512 + qi] = qi - 128*i2 - p
    qi = np.arange(512, dtype=np.float32)
    i2 = np.arange(2, dtype=np.float32)
    pch = np.arange(128, dtype=np.float32)
    qk_host = (
        (qi[None, None, :] - 128 * i2[None, :, None] - pch[:, None, None])
        .reshape(128, 1024)
        .astype(np.float16)
    )

    in_maps = []
    metas = []
    for c in range(8):
        b, t = c // 2, c % 2
        stripes = STRIPES_A if t == 0 else STRIPES_B
        eqT = np.concatenate(
            [encodings_for_q[b, st * 512 : (st + 1) * 512, :].T for st in stripes],
            axis=1,
        )
        ekT = encodings_for_k[b].T.reshape(256, 32, 128)
        ek_reord = np.concatenate([ekT[:, 0::2, :], ekT[:, 1::2, :]], axis=1).reshape(
            256, 4096
        )
        # thresholds: slot j exact if R[j] == T[j]
        thr = np.empty((16,), dtype=np.float16)
        for j in range(4):
            R = 4 * (stripes[j] + 1)
            vals = TH_EXACT if R == T[j] else TH_PAD
            thr[j * 4 : (j + 1) * 4] = vals
        thqk_host = np.ascontiguousarray(
            np.concatenate([np.broadcast_to(thr, (128, 16)), qk_host], axis=1),
            dtype=np.float16,
        )
        in_maps.append(
            {
                "eq": np.ascontiguousarray(eqT, dtype=bf),
                "ek": np.ascontiguousarray(ek_reord, dtype=bf),
                "ev": np.ascontiguousarray(encodings_for_v[b].T, dtype=bf),
                "wkqv": wkqv_host,
                "thqk": thqk_host,
            }
        )
        metas.append((b, stripes))

    res = run_bass_kernel_spmd(nc, in_maps, core_ids=list(range(8)))
    _CACHE["last_res"] = res

    out = np.empty((B, S, DM), dtype=np.float32)
    for c in range(8):
        b, stripes = metas[c]
        oT = res.results[c]["outT"].astype(np.float32)
        a = res.results[c]["acc"].astype(np.float32)
        for j, st in enumerate(stripes):
            r = a[:, j * 1024 : j * 1024 + 512].sum(0) + a[
                :, j * 1024 + 512 : (j + 1) * 1024
            ].sum(0)
            blk = oT[:, j * 512 : (j + 1) * 512] / r[None, :]
            out[b, st * 512 : (st + 1) * 512, :] = blk.T
    return out


# revision 44
# speedup vs baseline: 1.1560x; 1.0023x over previous
"""Causal attention head on 8 trn2 NeuronCores.

Sharding: core c = (batch b = c//2, type t = c%2). Each core handles 4
query stripes of 512 of its batch. Causal balance: type A gets stripes
[7,5,2,0] with real key-block counts R_A=[32,24,12,4]; type B stripes
[6,4,3,1] with R_B=[28,20,16,8]. One SPMD program: every core runs the
padded template T=[32,24,16,8]; per-core behaviour comes only from input
data (per-core threshold scalars select ones/triangle/zero mask tiles).

Everything on the PE array is bf16. Score matmuls pack two key-blocks
per issue via PE row tiling: kT pairs live on partition halves 0:64 /
64:128 (host interleaves ek into even/odd block regions), qT is
duplicated onto both halves by a column-duplicated Wq.

Attention processes pairs (2 key-blocks) in GROUPS of 2 to amortize the
PE stationary-switch stall (64-row score tiles vs 128-row pv tiles cost
~106ns per transition: drain + non-overlapped LDWEIGHTS). Per group g:
pv matmuls for group g-2 (8 MMs), then scores for the 2 pairs of group
g (psc double-buffer holds exactly 2 pairs), exp per pair on ACT, mask
mul (DVE) on the last-4 pairs, acc += e (DVE fp16).

The emission is software-pipelined ACROSS slots so the PE never idles at
a slot boundary: after slot j's last score group we emit [proj(p+1),
mask-gen(slot j+1), pv(ng-2)], and slot j+1's first two score groups
(which have no pv partner: GLAG=2) interleave with slot j's final pv
drain group and output copies. Input DMAs are split across the SP and
ACT issue queues with the critical slot-3 set (th/qk/wk/wq/ek/eq)
first; dummy matmuls keep the PE busy under the DMA wait so the HAM
clock-gate opens (1.2->2.4 GHz) early. QK (query-index minus key-index
iota) comes in as a host input so slot-3 masks only wait on two small
DMAs. Outputs: outT bf16 [256,2048] (unnormalized; o0 drained via DVE,
o1 via ACT in parallel), acc fp16 [128,4096]. Host: r = colsum(acc)
folded over pair halves; out = (outT/r).T.
"""

import sys

sys.path.insert(0, "/opt/trn_rl_repo")

import numpy as np
import ml_dtypes

B, S, DM, DQ = 4, 4096, 256, 64
T = [32, 24, 16, 8]  # padded template: key-blocks per slot
STRIPES_A = [7, 5, 2, 0]  # R_A = [32, 24, 12, 4]
STRIPES_B = [6, 4, 3, 1]  # R_B = [28, 20, 16, 8]
# Per-pair mask thresholds for the last 4 pairs of a slot (f16-safe:
# QK values lie in [-255, 511], so +-1000 mean all-ones / all-zeros).
# exact slot (R == T): pairs are [ones, ones, tri(0/128), tri(256/384)]
# padded slot (R == T-4): pairs are [tri(0/128), tri(256/384), zero, zero]
TH_EXACT = [-1000.0, -1000.0, 0.0, 256.0]
TH_PAD = [0.0, 256.0, 1000.0, 1000.0]

_CACHE = {}


def _build_nc():
    import concourse.bass as bass  # noqa: F401
    import concourse.tile as tile
    from concourse import bacc, mybir

    dt = mybir.dt
    f32, bf, f16 = dt.float32, dt.bfloat16, dt.float16

    nc = bacc.Bacc(
        "TRN2",
        target_bir_lowering=False,
        debug=False,
        enable_asserts=False,
        num_devices=8,
    )

    def din(name, shape, d):
        return nc.dram_tensor(name, shape, d, kind="ExternalInput").ap()

    eq = din("eq", [256, 2048], bf)
    ek = din("ek", [256, 4096], bf)  # column-reordered: even blocks, then odd
    ev = din("ev", [256, 4096], bf)
    # all projection weights in one tensor (one DMA per 128-partition half):
    # cols 0:64 Wk.T, 64:192 Wq.T duplicated, 192:448 Wv.T
    wkqv = din("wkqv", [256, 448], bf)
    # thresholds + QK iota in one f16 tensor: cols 0:16 th, 16:1040 QK
    # (QK[p, i2*512+qi] = qi-128*i2-p; th in {-1000,0,256,1000})
    thqk = din("thqk", [128, 1040], f16)
    outT = nc.dram_tensor("outT", [256, 2048], bf, kind="ExternalOutput").ap()
    acc_out = nc.dram_tensor("acc", [128, 4096], f16, kind="ExternalOutput").ap()

    Exp = mybir.ActivationFunctionType.Exp
    GE = mybir.AluOpType.is_ge

    with tile.TileContext(nc) as tc:
        from contextlib import ExitStack

        with ExitStack() as ctx:
            const = ctx.enter_context(tc.tile_pool(name="const", bufs=1))

            # ---- persistent SBUF tensors ----
            eq_sb = [const.tile([128, 2048], bf, tag=f"eq{h}", name=f"eq{h}") for h in range(2)]
            ek_sb = [const.tile([128, 4096], bf, tag=f"ek{h}", name=f"ek{h}") for h in range(2)]
            ev_sb = [const.tile([128, 4096], bf, tag=f"ev{h}", name=f"ev{h}") for h in range(2)]
            wkqv_sb = const.tile([128, 896], bf, tag="wkqv", name="wkqv")
            # per-half weight views into wkqv_sb
            wk_h = [wkqv_sb[:, h * 448 : h * 448 + 64] for h in range(2)]
            wq_h = [wkqv_sb[:, h * 448 + 64 : h * 448 + 192] for h in range(2)]
            wv_h = [wkqv_sb[:, h * 448 + 192 : h * 448 + 448] for h in range(2)]
            thqk_sb = const.tile([128, 1040], f16, tag="thqk", name="thqk")
            qk = thqk_sb[:, 16:1040]
            th_sb = const.tile([128, 16], f32, tag="th32", name="th32")
            qT = const.tile([128, 2048], bf, tag="qT", name="qT")  # dup halves
            kTp = const.tile([128, 2048], bf, tag="kTp", name="kTp")  # pair-packed
            v_sb = const.tile([128, 32 * 256], bf, tag="v", name="v")
            acc = const.tile([128, 4096], f16, tag="acc", name="acc")
            mk = const.tile([128, 16 * 1024], bf, tag="mk", name="mk")

            # Input DMAs are issued from both SP and Activation queues (half
            # each) and staged per phase: phases 0-1 up front, later phases
            # from inside the pipeline so issue time hides under compute.
            # Phase-0 order is criticality-driven: th/qk (mask gen), wk/wq,
            # ek (kTp proj), eq (qT proj), wv, ev.
            def dma_phase(p, j):
                # phases 0-1 are issued up front across FOUR queues (queue
                # issue+transfer serialize at ~1us per 128KB on one queue);
                # later phases go all-SP so ACT/DVE stay free for compute.
                cs_q = slice(j * 512, (j + 1) * 512)
                if p == 0:
                    nc.scalar.dma_start(thqk_sb[:], thqk[:])
                for reg in range(2):  # 0: even region, 1: odd region
                    cs = slice(reg * 2048 + p * 512, reg * 2048 + (p + 1) * 512)
                    if p == 0:
                        nc.sync.dma_start(ek_sb[0][:, cs], ek[0:128, cs])
                        nc.gpsimd.dma_start(ek_sb[1][:, cs], ek[128:256, cs])
                    elif p == 1:
                        nc.gpsimd.dma_start(ek_sb[0][:, cs], ek[0:128, cs])
                        nc.gpsimd.dma_start(ek_sb[1][:, cs], ek[128:256, cs])
                    else:
                        nc.sync.dma_start(ek_sb[0][:, cs], ek[0:128, cs])
                        nc.sync.dma_start(ek_sb[1][:, cs], ek[128:256, cs])
                if p == 0:
                    nc.sync.dma_start(wkqv_sb[:, 0:448], wkqv[0:128, :])
                    nc.scalar.dma_start(wkqv_sb[:, 448:896], wkqv[128:256, :])
                alt = nc.scalar if p == 0 else nc.sync
                nc.sync.dma_start(eq_sb[0][:, cs_q], eq[0:128, cs_q])
                alt.dma_start(eq_sb[1][:, cs_q], eq[128:256, cs_q])
                for cc in EV_CH[p]:
                    cs = slice(cc * 512, (cc + 1) * 512)
                    nc.sync.dma_start(ev_sb[0][:, cs], ev[0:128, cs])
                    alt.dma_start(ev_sb[1][:, cs], ev[128:256, cs])

            PHASES = ((0, 3), (1, 2), (2, 1), (3, 0))
            V_QUOTA = ((0, 1, 2, 3, 4, 5), (6, 7, 8, 9), (10, 11, 12, 13), (14, 15))
            EV_CH = ((0, 1, 2), (3, 4), (5, 6), (7,))
            # scr memset first: it must sit ahead of the phase-1 ek DMAs in
            # the GpSimd FIFO or the PE warm-up stalls behind 4 transfers
            scr = const.tile([128, 512], bf, tag="scr", name="scr")
            nc.gpsimd.memset(scr[:], 0.0)
            dma_phase(0, 3)
            dma_phase(1, 2)

            def mask_gen(j):
                # mk[t] = (QK >= th[t]) in bf16, on DVE (~478ns each; GpSimd's
                # tensor_scalar ucode is 30x slower for this op, do NOT use it)
                for m in range(4):
                    t = j * 4 + m
                    nc.vector.tensor_scalar(
                        mk[:, t * 1024 : (t + 1) * 1024], qk[:], th_sb[:, t : t + 1], None, GE
                    )

            # Slot-3 masks up front (its first score groups are all masked);
            # slot j+1's masks are generated during phase p's tail so the
            # 478ns-each DVE ops never sit ahead of critical qT/kTp casts.
            nc.vector.tensor_copy(th_sb[:], thqk_sb[:, 0:16])  # f16 -> f32
            mask_gen(3)

            pp = ctx.enter_context(tc.tile_pool(name="pp", bufs=2, space="PSUM"))
            psc = ctx.enter_context(tc.tile_pool(name="psc", bufs=2, space="PSUM"))
            po_pool = ctx.enter_context(tc.tile_pool(name="po", bufs=1, space="PSUM"))
            # 10 bufs: at a slot boundary 8 e-tiles are live at once (4
            # awaiting the previous slot's pv drain + 4 from the next
            # slot's first two score groups), so 8 can stall the exp stream
            epool = ctx.enter_context(tc.tile_pool(name="e", bufs=10))
            opool = ctx.enter_context(tc.tile_pool(name="o", bufs=2))

            def pp_tile():
                return pp.tile([128, 512], f32, tag="ps", name="ps")

            for _ in range(6):
                ps = pp_tile()
                nc.tensor.matmul(ps[:], scr[:, 0:128], scr[:], start=True, stop=True)

            def proj_qk(p, j):
                # kTp chunk p: even blocks -> partitions 0:64, odd -> 64:128
                # (the two 64-partition outputs col-tile and run concurrent)
                ps = pp_tile()
                for half in range(2):
                    dst = ps[half * 64 : (half + 1) * 64, :]
                    for h in range(2):
                        nc.tensor.matmul(
                            dst,
                            wk_h[h],
                            ek_sb[h][:, half * 2048 + p * 512 : half * 2048 + (p + 1) * 512],
                            start=(h == 0),
                            stop=(h == 1),
                        )
                nc.vector.tensor_copy(kTp[:, p * 512 : (p + 1) * 512], ps[:])
                # qT chunk j (duplicated onto both halves by the dup'd wq)
                ps = pp_tile()
                for h in range(2):
                    nc.tensor.matmul(
                        ps[:],
                        wq_h[h],
                        eq_sb[h][:, j * 512 : (j + 1) * 512],
                        start=(h == 0),
                        stop=(h == 1),
                    )
                nc.vector.tensor_copy(qT[:, j * 512 : (j + 1) * 512], ps[:])

            def v_proj(tiles):
                # v pairs (natural [keys, 256] bf16); copies go 2-of-3 on
                # DVE, 1-of-3 on ACT so ACT keeps the exp cadence
                for n, i in enumerate(tiles):
                    ps = pp_tile()
                    for s in range(2):
                        t = 2 * i + s
                        for h in range(2):
                            nc.tensor.matmul(
                                ps[:, s * 256 : (s + 1) * 256],
                                ev_sb[h][:, t * 128 : (t + 1) * 128],
                                wv_h[h],
                                start=(h == 0),
                                stop=(h == 1),
                            )
                    dst = v_sb[:, i * 512 : (i + 1) * 512]
                    # 1-of-3 on ACT: more ACT copies break the back-to-back
                    # exp pipelining, more DVE copies choke the acc chain
                    if n % 3 == 2:
                        nc.scalar.copy(dst, ps[:])
                    else:
                        nc.vector.tensor_copy(dst, ps[:])

            G, GLAG = 2, 2  # score-pair group size, pv lag in groups

            class Slot:
                def __init__(self, p, j):
                    self.p, self.j = p, j
                    self.npairs = T[j] // 2
                    self.ngroups = self.npairs // G
                    self.po0 = po_pool.tile([128, 512], f32, tag="po0", name="po0")
                    self.po1 = po_pool.tile([128, 512], f32, tag="po1", name="po1")
                    self.qs_top = qT[0:64, j * 512 : (j + 1) * 512]
                    self.qs_bot = qT[64:128, j * 512 : (j + 1) * 512]
                    self.acc_j = acc[:, j * 1024 : (j + 1) * 1024]
                    self.es = [None] * self.npairs
                    # Process masked pairs right after the pipeline fills
                    # (their DVE mask-muls get full lag slack) and end every
                    # slot on unmasked pairs so the drain has no DVE dep.
                    np_ = self.npairs
                    if np_ > 4:
                        k = min(4, np_ - 8)
                        self.proc = (
                            list(range(k)) + list(range(np_ - 4, np_)) + list(range(k, np_ - 4))
                        )
                    else:
                        self.proc = list(range(np_))

                def pv_group(self, h):
                    # 4 pv matmuls per pair, 8 per group; LDWEIGHTS chains
                    # through the background weight buffer at 216ns cadence.
                    # (64-row pv splits were tried twice: shared-bank
                    # interleave hangs the device; per-pipe banks work and
                    # reach ~1070ns/pair steady BUT need the pp pool's 2
                    # PSUM banks, and proj tiles rotating through psc
                    # serialize slot boundaries for a net loss. Keep 128-row.)
                    for dd in range(h * G, h * G + G):
                        e = self.es[dd]
                        for half in range(2):
                            u = 2 * self.proc[dd] + half
                            eh = e[:, half * 512 : (half + 1) * 512]
                            first = dd == 0 and half == 0
                            last = dd == self.npairs - 1 and half == 1
                            nc.tensor.matmul(
                                self.po0[:],
                                v_sb[:, u * 256 : u * 256 + 128],
                                eh,
                                start=first,
                                stop=last,
                            )
                            nc.tensor.matmul(
                                self.po1[:],
                                v_sb[:, u * 256 + 128 : (u + 1) * 256],
                                eh,
                                start=first,
                                stop=last,
                            )

                def sc_group(self, g):
                    # scores for the G pairs of group g, then exp/mask/acc
                    for d in range(g * G, g * G + G):
                        tp = self.proc[d]
                        ps = psc.tile([128, 1024], f32, tag="ps", name="ps")
                        nc.tensor.matmul(
                            ps[:, 0:512],
                            kTp[0:64, tp * 128 : (tp + 1) * 128],
                            self.qs_top,
                            start=True,
                            stop=True,
                        )
                        nc.tensor.matmul(
                            ps[:, 512:1024],
                            kTp[64:128, tp * 128 : (tp + 1) * 128],
                            self.qs_bot,
                            start=True,
                            stop=True,
                        )
                        e = epool.tile([128, 1024], bf, tag="e", name="e")
                        nc.scalar.activation(e[:], ps[:], Exp, scale=0.125)
                        if tp >= self.npairs - 4:
                            t = self.j * 4 + tp - (self.npairs - 4)
                            nc.vector.tensor_mul(
                                e[:], e[:], mk[:, t * 1024 : (t + 1) * 1024]
                            )
                        if d == 0:
                            nc.vector.tensor_copy(self.acc_j, e[:])
                        else:
                            nc.vector.tensor_add(self.acc_j, self.acc_j, e[:])
                        self.es[d] = e

                def fin(self, last_phase):
                    nc.sync.dma_start(
                        acc_out[:, self.j * 1024 : (self.j + 1) * 1024], self.acc_j
                    )
                    o0 = opool.tile([128, 512], bf, tag="o0", name="o0")
                    o1 = opool.tile([128, 512], bf, tag="o1", name="o1")
                    # o0 via DVE, o1 via ACT: the drains run in parallel and
                    # ACT never blocks long behind the pv-drain semaphore.
                    nc.vector.tensor_copy(o0[:], self.po0[:])
                    nc.scalar.copy(o1[:], self.po1[:])
                    j = self.j
                    nc.sync.dma_start(outT[0:128, j * 512 : (j + 1) * 512], o0[:])
                    if last_phase:  # final slot: second out DMA from ACT queue
                        nc.scalar.dma_start(outT[128:256, j * 512 : (j + 1) * 512], o1[:])
                    else:
                        nc.sync.dma_start(outT[128:256, j * 512 : (j + 1) * 512], o1[:])

            # ---- software-pipelined emission across phases ----
            # Per phase p (slot j): [sc0, prev.pv(ng-1), sc1, prev.fin,
            # v_proj(p), {pv(g-2), sc(g)}..., proj(p+1), mask_gen(j+1),
            # pv(ng-2)]; the final pv drain and output copies of slot j land
            # inside phase p+1's head, overlapped with its first exps.
            proj_qk(0, 3)
            prev = None
            for p, j in PHASES:
                s = Slot(p, j)
                s.sc_group(0)
                if prev is not None:
                    prev.pv_group(prev.ngroups - 1)
                s.sc_group(1)
                if prev is not None:
                    prev.fin(last_phase=False)
                # phase 0: only tiles 0-1 are needed before pv(0); tiles 2-3
                # are emitted in the tail so the first exps start sooner
                v_proj(V_QUOTA[p][:2] if p == 0 else V_QUOTA[p])
                for g in range(2, s.ngroups):
                    s.pv_group(g - 2)
                    s.sc_group(g)
                if p < 3:
                    proj_qk(p + 1, PHASES[p + 1][1])
                    mask_gen(PHASES[p + 1][1])
                if p == 0:
                    v_proj(V_QUOTA[0][2:])
                s.pv_group(s.ngroups - 2)
                if p + 2 <= 3:
                    dma_phase(*PHASES[p + 2])
                prev = s
            prev.pv_group(prev.ngroups - 1)
            prev.fin(last_phase=True)

    nc.compile()
    return nc


def kernel(encodings_for_q, encodings_for_k, encodings_for_v, mask, Wq, Wk, Wv):
    from concourse.bass_utils import run_bass_kernel_spmd

    if "nc" not in _CACHE:
        _CACHE["nc"] = _build_nc()
    nc = _CACHE["nc"]

    bf = ml_dtypes.bfloat16
    # [Wk.T | Wq.T dup | Wv.T] -> [256, 448]
    wkqv_host = np.ascontiguousarray(
        np.concatenate([Wk.T, Wq.T, Wq.T, Wv.T], axis=1), dtype=bf
    )
    # QK[p, i2*512 + qi] = qi - 128*i2 - p
    qi = np.arange(512, dtype=np.float32)
    i2 = np.arange(2, dtype=np.float32)
    pch = np.arange(128, dtype=np.float32)
    qk_host = (
        (qi[None, None, :] - 128 * i2[None, :, None] - pch[:, None, None])
        .reshape(128, 1024)
        .astype(np.float16)
    )

    in_maps = []
    metas = []
    for c in range(8):
        b, t = c // 2, c % 2
        stripes = STRIPES_A if t == 0 else STRIPES_B
        eqT = np.concatenate(
            [encodings_for_q[b, st * 512 : (st + 1) * 512, :].T for st in stripes],
            axis=1,
        )
        ekT = encodings_for_k[b].T.reshape(256, 32, 128)
        ek_reord = np.concatenate([ekT[:, 0::2, :], ekT[:, 1::2, :]], axis=1).reshape(
            256, 4096
        )
        # thresholds: slot j exact if R[j] == T[j]
        thr = np.empty((16,), dtype=np.float16)
        for j in range(4):
            R = 4 * (stripes[j] + 1)
            vals = TH_EXACT if R == T[j] else TH_PAD
            thr[j * 4 : (j + 1) * 4] = vals
        thqk_host = np.ascontiguousarray(
            np.concatenate([np.broadcast_to(thr, (128, 16)), qk_host], axis=1),
            dtype=np.float16,
        )
        in_maps.append(
            {
                "eq": np.ascontiguousarray(eqT, dtype=bf),
                "ek": np.ascontiguousarray(ek_reord, dtype=bf),
                "ev": np.ascontiguousarray(encodings_for_v[b].T, dtype=bf),
                "wkqv": wkqv_host,
                "thqk": thqk_host,
            }
        )
        metas.append((b, stripes))

    res = run_bass_kernel_spmd(nc, in_maps, core_ids=list(range(8)))
    _CACHE["last_res"] = res

    out = np.empty((B, S, DM), dtype=np.float32)
    for c in range(8):
        b, stripes = metas[c]
        oT = res.results[c]["outT"].astype(np.float32)
        a = res.results[c]["acc"].astype(np.float32)
        for j, st in enumerate(stripes):
            r = a[:, j * 1024 : j * 1024 + 512].sum(0) + a[
                :, j * 1024 + 512 : (j + 1) * 1024
            ].sum(0)
            blk = oT[:, j * 512 : (j + 1) * 512] / r[None, :]
            out[b, st * 512 : (st + 1) * 512, :] = blk.T
    return out


# revision 45
# speedup vs baseline: 1.1622x; 1.0054x over previous
"""Causal attention head on 8 trn2 NeuronCores.

Sharding: core c = (batch b = c//2, type t = c%2). Each core handles 4
query stripes of 512 of its batch. Causal balance: type A gets stripes
[7,5,2,0] with real key-block counts R_A=[32,24,12,4]; type B stripes
[6,4,3,1] with R_B=[28,20,16,8]. One SPMD program: every core runs the
padded template T=[32,24,16,8]; per-core behaviour comes only from input
data (per-core threshold scalars select ones/triangle/zero mask tiles).

Everything on the PE array is bf16. Score matmuls pack two key-blocks
per issue via PE row tiling: kT pairs live on partition halves 0:64 /
64:128 (host interleaves ek into even/odd block regions), qT is
duplicated onto both halves by a column-duplicated Wq.

Attention processes pairs (2 key-blocks) in GROUPS of 2 to amortize the
PE stationary-switch stall (64-row score tiles vs 128-row pv tiles cost
~106ns per transition: drain + non-overlapped LDWEIGHTS). Per group g:
pv matmuls for group g-2 (8 MMs), then scores for the 2 pairs of group
g (psc double-buffer holds exactly 2 pairs), exp per pair on ACT, mask
mul (DVE) on the last-4 pairs, acc += e (DVE fp16).

The emission is software-pipelined ACROSS slots so the PE never idles at
a slot boundary: after slot j's last score group we emit [proj(p+1),
mask-gen(slot j+1), pv(ng-2)], and slot j+1's first two score groups
(which have no pv partner: GLAG=2) interleave with slot j's final pv
drain group and output copies. Input DMAs are split across the SP and
ACT issue queues with the critical slot-3 set (th/qk/wk/wq/ek/eq)
first; dummy matmuls keep the PE busy under the DMA wait so the HAM
clock-gate opens (1.2->2.4 GHz) early. QK (query-index minus key-index
iota) comes in as a host input so slot-3 masks only wait on two small
DMAs. Outputs: outT bf16 [256,2048] (unnormalized; o0 drained via DVE,
o1 via ACT in parallel), acc fp16 [128,4096]. Host: r = colsum(acc)
folded over pair halves; out = (outT/r).T.
"""

import sys

sys.path.insert(0, "/opt/trn_rl_repo")

import numpy as np
import ml_dtypes

B, S, DM, DQ = 4, 4096, 256, 64
T = [32, 24, 16, 8]  # padded template: key-blocks per slot
STRIPES_A = [7, 5, 2, 0]  # R_A = [32, 24, 12, 4]
STRIPES_B = [6, 4, 3, 1]  # R_B = [28, 20, 16, 8]
# Per-pair mask thresholds for the last 4 pairs of a slot (f16-safe:
# QK values lie in [-255, 511], so +-1000 mean all-ones / all-zeros).
# exact slot (R == T): pairs are [ones, ones, tri(0/128), tri(256/384)]
# padded slot (R == T-4): pairs are [tri(0/128), tri(256/384), zero, zero]
TH_EXACT = [-1000.0, -1000.0, 0.0, 256.0]
TH_PAD = [0.0, 256.0, 1000.0, 1000.0]

_CACHE = {}


def _build_nc():
    import concourse.bass as bass  # noqa: F401
    import concourse.tile as tile
    from concourse import bacc, mybir

    dt = mybir.dt
    f32, bf, f16 = dt.float32, dt.bfloat16, dt.float16

    nc = bacc.Bacc(
        "TRN2",
        target_bir_lowering=False,
        debug=False,
        enable_asserts=False,
        num_devices=8,
    )

    def din(name, shape, d):
        return nc.dram_tensor(name, shape, d, kind="ExternalInput").ap()

    eq = din("eq", [256, 2048], bf)
    ek = din("ek", [256, 4096], bf)  # column-reordered: even blocks, then odd
    ev = din("ev", [256, 4096], bf)
    # all projection weights in one tensor (one DMA per 128-partition half):
    # cols 0:64 Wk.T, 64:192 Wq.T duplicated, 192:448 Wv.T
    wkqv = din("wkqv", [256, 448], bf)
    # thresholds + QK iota in one f16 tensor: cols 0:16 th, 16:1040 QK
    # (QK[p, i2*512+qi] = qi-128*i2-p; th in {-1000,0,256,1000})
    thqk = din("thqk", [128, 1040], f16)
    outT = nc.dram_tensor("outT", [256, 2048], bf, kind="ExternalOutput").ap()
    acc_out = nc.dram_tensor("acc", [128, 4096], f16, kind="ExternalOutput").ap()

    Exp = mybir.ActivationFunctionType.Exp
    GE = mybir.AluOpType.is_ge

    with tile.TileContext(nc) as tc:
        from contextlib import ExitStack

        with ExitStack() as ctx:
            const = ctx.enter_context(tc.tile_pool(name="const", bufs=1))

            # ---- persistent SBUF tensors ----
            eq_sb = [const.tile([128, 2048], bf, tag=f"eq{h}", name=f"eq{h}") for h in range(2)]
            ek_sb = [const.tile([128, 4096], bf, tag=f"ek{h}", name=f"ek{h}") for h in range(2)]
            ev_sb = [const.tile([128, 4096], bf, tag=f"ev{h}", name=f"ev{h}") for h in range(2)]
            wkqv_sb = const.tile([128, 896], bf, tag="wkqv", name="wkqv")
            # per-half weight views into wkqv_sb
            wk_h = [wkqv_sb[:, h * 448 : h * 448 + 64] for h in range(2)]
            wq_h = [wkqv_sb[:, h * 448 + 64 : h * 448 + 192] for h in range(2)]
            wv_h = [wkqv_sb[:, h * 448 + 192 : h * 448 + 448] for h in range(2)]
            thqk_sb = const.tile([128, 1040], f16, tag="thqk", name="thqk")
            qk = thqk_sb[:, 16:1040]
            th_sb = const.tile([128, 16], f32, tag="th32", name="th32")
            qT = const.tile([128, 2048], bf, tag="qT", name="qT")  # dup halves
            kTp = const.tile([128, 2048], bf, tag="kTp", name="kTp")  # pair-packed
            v_sb = const.tile([128, 32 * 256], bf, tag="v", name="v")
            acc = const.tile([128, 4096], f16, tag="acc", name="acc")
            mk = const.tile([128, 16 * 1024], bf, tag="mk", name="mk")

            # Input DMAs are issued from both SP and Activation queues (half
            # each) and staged per phase: phases 0-1 up front, later phases
            # from inside the pipeline so issue time hides under compute.
            # Phase-0 order is criticality-driven: th/qk (mask gen), wk/wq,
            # ek (kTp proj), eq (qT proj), wv, ev.
            def dma_phase(p, j):
                # phases 0-1 are issued up front across FOUR queues (queue
                # issue+transfer serialize at ~1us per 128KB on one queue);
                # later phases go all-SP so ACT/DVE stay free for compute.
                cs_q = slice(j * 512, (j + 1) * 512)
                if p == 0:
                    nc.scalar.dma_start(thqk_sb[:], thqk[:])
                for reg in range(2):  # 0: even region, 1: odd region
                    cs = slice(reg * 2048 + p * 512, reg * 2048 + (p + 1) * 512)
                    if p == 0:
                        nc.sync.dma_start(ek_sb[0][:, cs], ek[0:128, cs])
                        nc.gpsimd.dma_start(ek_sb[1][:, cs], ek[128:256, cs])
                    elif p == 1:
                        nc.gpsimd.dma_start(ek_sb[0][:, cs], ek[0:128, cs])
                        nc.gpsimd.dma_start(ek_sb[1][:, cs], ek[128:256, cs])
                    else:
                        nc.sync.dma_start(ek_sb[0][:, cs], ek[0:128, cs])
                        nc.sync.dma_start(ek_sb[1][:, cs], ek[128:256, cs])
                if p == 0:
                    nc.sync.dma_start(wkqv_sb[:, 0:448], wkqv[0:128, :])
                    nc.scalar.dma_start(wkqv_sb[:, 448:896], wkqv[128:256, :])
                alt = nc.scalar if p == 0 else nc.sync
                nc.sync.dma_start(eq_sb[0][:, cs_q], eq[0:128, cs_q])
                alt.dma_start(eq_sb[1][:, cs_q], eq[128:256, cs_q])
                for cc in EV_CH[p]:
                    cs = slice(cc * 512, (cc + 1) * 512)
                    nc.sync.dma_start(ev_sb[0][:, cs], ev[0:128, cs])
                    alt.dma_start(ev_sb[1][:, cs], ev[128:256, cs])

            PHASES = ((0, 3), (1, 2), (2, 1), (3, 0))
            V_QUOTA = ((0, 1, 2, 3, 4, 5), (6, 7, 8, 9), (10, 11, 12, 13), (14, 15))
            EV_CH = ((0, 1, 2), (3, 4), (5, 6), (7,))
            # scr memset first: it must sit ahead of the phase-1 ek DMAs in
            # the GpSimd FIFO or the PE warm-up stalls behind 4 transfers
            scr = const.tile([128, 512], bf, tag="scr", name="scr")
            nc.gpsimd.memset(scr[:], 0.0)
            dma_phase(0, 3)
            dma_phase(1, 2)

            def mask_gen(j):
                # mk[t] = (QK >= th[t]) in bf16, on DVE (~478ns each; GpSimd's
                # tensor_scalar ucode is 30x slower for this op, do NOT use it)
                for m in range(4):
                    t = j * 4 + m
                    nc.vector.tensor_scalar(
                        mk[:, t * 1024 : (t + 1) * 1024], qk[:], th_sb[:, t : t + 1], None, GE
                    )

            # Slot-3 masks up front (its first score groups are all masked);
            # slot j+1's masks are generated during phase p's tail so the
            # 478ns-each DVE ops never sit ahead of critical qT/kTp casts.
            nc.vector.tensor_copy(th_sb[:], thqk_sb[:, 0:16])  # f16 -> f32
            mask_gen(3)

            pp = ctx.enter_context(tc.tile_pool(name="pp", bufs=2, space="PSUM"))
            psc = ctx.enter_context(tc.tile_pool(name="psc", bufs=2, space="PSUM"))
            po_pool = ctx.enter_context(tc.tile_pool(name="po", bufs=1, space="PSUM"))
            # 10 bufs: at a slot boundary 8 e-tiles are live at once (4
            # awaiting the previous slot's pv drain + 4 from the next
            # slot's first two score groups), so 8 can stall the exp stream
            epool = ctx.enter_context(tc.tile_pool(name="e", bufs=10))
            opool = ctx.enter_context(tc.tile_pool(name="o", bufs=2))

            def pp_tile():
                return pp.tile([128, 512], f32, tag="ps", name="ps")

            for _ in range(6):
                ps = pp_tile()
                nc.tensor.matmul(ps[:], scr[:, 0:128], scr[:], start=True, stop=True)

            def proj_qk(p, j):
                # kTp chunk p: even blocks -> partitions 0:64, odd -> 64:128
                # (the two 64-partition outputs col-tile and run concurrent)
                ps = pp_tile()
                for half in range(2):
                    dst = ps[half * 64 : (half + 1) * 64, :]
                    for h in range(2):
                        nc.tensor.matmul(
                            dst,
                            wk_h[h],
                            ek_sb[h][:, half * 2048 + p * 512 : half * 2048 + (p + 1) * 512],
                            start=(h == 0),
                            stop=(h == 1),
                        )
                nc.vector.tensor_copy(kTp[:, p * 512 : (p + 1) * 512], ps[:])
                # qT chunk j (duplicated onto both halves by the dup'd wq)
                ps = pp_tile()
                for h in range(2):
                    nc.tensor.matmul(
                        ps[:],
                        wq_h[h],
                        eq_sb[h][:, j * 512 : (j + 1) * 512],
                        start=(h == 0),
                        stop=(h == 1),
                    )
                nc.vector.tensor_copy(qT[:, j * 512 : (j + 1) * 512], ps[:])

            def v_proj(tiles):
                # v pairs (natural [keys, 256] bf16); copies go 2-of-3 on
                # DVE, 1-of-3 on ACT so ACT keeps the exp cadence
                for n, i in enumerate(tiles):
                    ps = pp_tile()
                    for s in range(2):
                        t = 2 * i + s
                        for h in range(2):
                            nc.tensor.matmul(
                                ps[:, s * 256 : (s + 1) * 256],
                                ev_sb[h][:, t * 128 : (t + 1) * 128],
                                wv_h[h],
                                start=(h == 0),
                                stop=(h == 1),
                            )
                    dst = v_sb[:, i * 512 : (i + 1) * 512]
                    # 1-of-3 on ACT: more ACT copies break the back-to-back
                    # exp pipelining, more DVE copies choke the acc chain
                    if n % 3 == 2:
                        nc.scalar.copy(dst, ps[:])
                    else:
                        nc.vector.tensor_copy(dst, ps[:])

            G, GLAG = 2, 2  # score-pair group size, pv lag in groups

            class Slot:
                def __init__(self, p, j):
                    self.p, self.j = p, j
                    self.npairs = T[j] // 2
                    self.ngroups = self.npairs // G
                    self.po0 = po_pool.tile([128, 512], f32, tag="po0", name="po0")
                    self.po1 = po_pool.tile([128, 512], f32, tag="po1", name="po1")
                    self.qs_top = qT[0:64, j * 512 : (j + 1) * 512]
                    self.qs_bot = qT[64:128, j * 512 : (j + 1) * 512]
                    self.acc_j = acc[:, j * 1024 : (j + 1) * 1024]
                    self.es = [None] * self.npairs
                    # Process masked pairs right after the pipeline fills
                    # (their DVE mask-muls get full lag slack) and end every
                    # slot on unmasked pairs so the drain has no DVE dep.
                    np_ = self.npairs
                    if np_ > 4:
                        k = min(4, np_ - 8)
                        self.proc = (
                            list(range(k)) + list(range(np_ - 4, np_)) + list(range(k, np_ - 4))
                        )
                    else:
                        self.proc = list(range(np_))

                def pv_group(self, h):
                    # 4 pv matmuls per pair, 8 per group; LDWEIGHTS chains
                    # through the background weight buffer at 216ns cadence.
                    # (64-row pv splits were tried twice: shared-bank
                    # interleave hangs the device; per-pipe banks work and
                    # reach ~1070ns/pair steady BUT need the pp pool's 2
                    # PSUM banks, and proj tiles rotating through psc
                    # serialize slot boundaries for a net loss. Keep 128-row.)
                    if self.j == 0 and h == self.ngroups - 1:
                        # kernel-final drain group: all po0 matmuls first so
                        # po0 stops ~0.9us earlier and o0's copy + outT DMA
                        # overlap the po1 tail (start is never needed here)
                        for bank in range(2):
                            po = self.po0 if bank == 0 else self.po1
                            for dd in range(h * G, h * G + G):
                                e = self.es[dd]
                                for half in range(2):
                                    u = 2 * self.proc[dd] + half
                                    cs = (
                                        slice(u * 256, u * 256 + 128)
                                        if bank == 0
                                        else slice(u * 256 + 128, (u + 1) * 256)
                                    )
                                    nc.tensor.matmul(
                                        po[:],
                                        v_sb[:, cs],
                                        e[:, half * 512 : (half + 1) * 512],
                                        start=False,
                                        stop=dd == self.npairs - 1 and half == 1,
                                    )
                        return
                    for dd in range(h * G, h * G + G):
                        e = self.es[dd]
                        for half in range(2):
                            u = 2 * self.proc[dd] + half
                            eh = e[:, half * 512 : (half + 1) * 512]
                            first = dd == 0 and half == 0
                            last = dd == self.npairs - 1 and half == 1
                            nc.tensor.matmul(
                                self.po0[:],
                                v_sb[:, u * 256 : u * 256 + 128],
                                eh,
                                start=first,
                                stop=last,
                            )
                            nc.tensor.matmul(
                                self.po1[:],
                                v_sb[:, u * 256 + 128 : (u + 1) * 256],
                                eh,
                                start=first,
                                stop=last,
                            )

                def sc_group(self, g):
                    # scores for the G pairs of group g, then exp/mask/acc
                    for d in range(g * G, g * G + G):
                        tp = self.proc[d]
                        ps = psc.tile([128, 1024], f32, tag="ps", name="ps")
                        nc.tensor.matmul(
                            ps[:, 0:512],
                            kTp[0:64, tp * 128 : (tp + 1) * 128],
                            self.qs_top,
                            start=True,
                            stop=True,
                        )
                        nc.tensor.matmul(
                            ps[:, 512:1024],
                            kTp[64:128, tp * 128 : (tp + 1) * 128],
                            self.qs_bot,
                            start=True,
                            stop=True,
                        )
                        e = epool.tile([128, 1024], bf, tag="e", name="e")
                        nc.scalar.activation(e[:], ps[:], Exp, scale=0.125)
                        if tp >= self.npairs - 4:
                            t = self.j * 4 + tp - (self.npairs - 4)
                            nc.vector.tensor_mul(
                                e[:], e[:], mk[:, t * 1024 : (t + 1) * 1024]
                            )
                        if d == 0:
                            nc.vector.tensor_copy(self.acc_j, e[:])
                        else:
                            nc.vector.tensor_add(self.acc_j, self.acc_j, e[:])
                        self.es[d] = e

                def fin(self, last_phase):
                    nc.sync.dma_start(
                        acc_out[:, self.j * 1024 : (self.j + 1) * 1024], self.acc_j
                    )
                    o0 = opool.tile([128, 512], bf, tag="o0", name="o0")
                    o1 = opool.tile([128, 512], bf, tag="o1", name="o1")
                    # o0 via DVE, o1 via ACT: the drains run in parallel and
                    # ACT never blocks long behind the pv-drain semaphore.
                    nc.vector.tensor_copy(o0[:], self.po0[:])
                    nc.scalar.copy(o1[:], self.po1[:])
                    j = self.j
                    nc.sync.dma_start(outT[0:128, j * 512 : (j + 1) * 512], o0[:])
                    if last_phase:  # final slot: second out DMA from ACT queue
                        nc.scalar.dma_start(outT[128:256, j * 512 : (j + 1) * 512], o1[:])
                    else:
                        nc.sync.dma_start(outT[128:256, j * 512 : (j + 1) * 512], o1[:])

            # ---- software-pipelined emission across phases ----
            # Per phase p (slot j): [sc0, prev.pv(ng-1), sc1, prev.fin,
            # v_proj(p), {pv(g-2), sc(g)}..., proj(p+1), mask_gen(j+1),
            # pv(ng-2)]; the final pv drain and output copies of slot j land
            # inside phase p+1's head, overlapped with its first exps.
            proj_qk(0, 3)
            prev = None
            for p, j in PHASES:
                s = Slot(p, j)
                s.sc_group(0)
                if prev is not None:
                    prev.pv_group(prev.ngroups - 1)
                s.sc_group(1)
                if prev is not None:
                    prev.fin(last_phase=False)
                # phase 0: only tiles 0-1 are needed before pv(0); tiles 2-3
                # are emitted in the tail so the first exps start sooner
                v_proj(V_QUOTA[p][:2] if p == 0 else V_QUOTA[p])
                for g in range(2, s.ngroups):
                    s.pv_group(g - 2)
                    s.sc_group(g)
                if p < 3:
                    proj_qk(p + 1, PHASES[p + 1][1])
                    mask_gen(PHASES[p + 1][1])
                if p == 0:
                    v_proj(V_QUOTA[0][2:])
                s.pv_group(s.ngroups - 2)
                if p + 2 <= 3:
                    dma_phase(*PHASES[p + 2])
                prev = s
            prev.pv_group(prev.ngroups - 1)
            prev.fin(last_phase=True)

    nc.compile()
    return nc


def kernel(encodings_for_q, encodings_for_k, encodings_for_v, mask, Wq, Wk, Wv):
    from concourse.bass_utils import run_bass_kernel_spmd

    if "nc" not in _CACHE:
        _CACHE["nc"] = _build_nc()
    nc = _CACHE["nc"]

    bf = ml_dtypes.bfloat16
    # [Wk.T | Wq.T dup | Wv.T] -> [256, 448]
    wkqv_host = np.ascontiguousarray(
        np.concatenate([Wk.T, Wq.T, Wq.T, Wv.T], axis=1), dtype=bf
    )
    # QK[p, i2*512 + qi] = qi - 128*i2 - p
    qi = np.arange(512, dtype=np.float32)
    i2 = np.arange(2, dtype=np.float32)
    pch = np.arange(128, dtype=np.float32)
    qk_host = (
        (qi[None, None, :] - 128 * i2[None, :, None] - pch[:, None, None])
        .reshape(128, 1024)
        .astype(np.float16)
    )

    in_maps = []
    metas = []
    for c in range(8):
        b, t = c // 2, c % 2
        stripes = STRIPES_A if t == 0 else STRIPES_B
        eqT = np.concatenate(
            [encodings_for_q[b, st * 512 : (st + 1) * 512, :].T for st in stripes],
            axis=1,
        )
        ekT = encodings_for_k[b].T.reshape(256, 32, 128)
        ek_reord = np.concatenate([ekT[:, 0::2, :], ekT[:, 1::2, :]], axis=1).reshape(
            256, 4096
        )
        # thresholds: slot j exact if R[j] == T[j]
        thr = np.empty((16,), dtype=np.float16)
        for j in range(4):
            R = 4 * (stripes[j] + 1)
            vals = TH_EXACT if R == T[j] else TH_PAD
            thr[j * 4 : (j + 1) * 4] = vals
        thqk_host = np.ascontiguousarray(
            np.concatenate([np.broadcast_to(thr, (128, 16)), qk_host], axis=1),
            dtype=np.float16,
        )
        in_maps.append(
            {
                "eq": np.ascontiguousarray(eqT, dtype=bf),
                "ek": np.ascontiguousarray(ek_reord, dtype=bf),
                "ev": np.ascontiguousarray(encodings_for_v[b].T, dtype=bf),
                "wkqv": wkqv_host,
                "thqk": thqk_host,
            }
        )
        metas.append((b, stripes))

    res = run_bass_kernel_spmd(nc, in_maps, core_ids=list(range(8)))
    _CACHE["last_res"] = res

    out = np.empty((B, S, DM), dtype=np.float32)
    for c in range(8):
        b, stripes = metas[c]
        oT = res.results[c]["outT"].astype(np.float32)
        a = res.results[c]["acc"].astype(np.float32)
        for j, st in enumerate(stripes):
            r = a[:, j * 1024 : j * 1024 + 512].sum(0) + a[
                :, j * 1024 + 512 : (j + 1) * 1024
            ].sum(0)
            blk = oT[:, j * 512 : (j + 1) * 512] / r[None, :]
            out[b, st * 512 : (st + 1) * 512, :] = blk.T
    return out
